# revision 5
# baseline (speedup 1.0000x reference)
"""Trainium2 Bass kernel for DigitConvolutionalModel.

Pipeline (per core, pure data-parallel over batch):
  x [8192, 784] --DMA--> SBUF batch-major --PE transpose--> feature-major tiles
  conv 3x3 as banded block-matmuls on PE -> relu -> fc1 (matmul) -> relu
  -> fc2 (matmul) + bias -> DMA out.

All activations live feature-major ([features, batch]) so the PE can contract
over the partition dim. The 3x3 conv is expressed as 13 small banded matmuls
per 512-batch tile using three constant band matrices built on the host from
conv_w (shift-invariant across 4-image-row blocks).
"""

import numpy as np
from contextlib import ExitStack

N_CORES = 8
B_FULL = 65536
B_CORE = B_FULL // N_CORES  # 8192
BT = 512                    # batch tile (matmul moving free dim)
NT = B_CORE // BT           # 16

_cache = {}


def _build_module(b_core=B_CORE, n_cores=N_CORES):
    import concourse.bass as bass
    import concourse.tile as tile
    from concourse import bacc, mybir

    f32 = mybir.dt.float32
    f32r = mybir.dt.float32r
    AF = mybir.ActivationFunctionType
    nt = b_core // BT

    nc = bacc.Bacc("TRN2", target_bir_lowering=False, debug=False,
                   num_devices=n_cores)

    x_d = nc.dram_tensor("x", [b_core, 784], f32, kind="ExternalInput").ap()
    kA_d = nc.dram_tensor("kA", [112, 104], f32, kind="ExternalInput").ap()
    kB_d = nc.dram_tensor("kB", [56, 104], f32, kind="ExternalInput").ap()
    kC_d = nc.dram_tensor("kC", [112, 52], f32, kind="ExternalInput").ap()
    w1_d = nc.dram_tensor("w1", [676, 128], f32, kind="ExternalInput").ap()
    b1_d = nc.dram_tensor("b1", [128, 1], f32, kind="ExternalInput").ap()
    w2_d = nc.dram_tensor("w2", [128, 10], f32, kind="ExternalInput").ap()
    b2_d = nc.dram_tensor("b2", [10, 1], f32, kind="ExternalInput").ap()
    id_d = nc.dram_tensor("iden", [128, 128], f32, kind="ExternalInput").ap()
    y_d = nc.dram_tensor("y", [10, b_core], f32, kind="ExternalOutput").ap()

    with tile.TileContext(nc) as tc, ExitStack() as ctx:
        const = ctx.enter_context(tc.tile_pool(name="const", bufs=1))
        xbm_p = ctx.enter_context(tc.tile_pool(name="xbm", bufs=4))
        xfm_p = ctx.enter_context(tc.tile_pool(name="xfm", bufs=21))
        h_p = ctx.enter_context(tc.tile_pool(name="h", bufs=14))
        h1_p = ctx.enter_context(tc.tile_pool(name="h1", bufs=2))
        o_p = ctx.enter_context(tc.tile_pool(name="osb", bufs=2))
        tp_ps = ctx.enter_context(tc.tile_pool(name="tp_ps", bufs=2, space="PSUM"))
        cv_ps = ctx.enter_context(tc.tile_pool(name="cv_ps", bufs=2, space="PSUM"))
        f1_ps = ctx.enter_context(tc.tile_pool(name="f1_ps", bufs=2, space="PSUM"))
        f2_ps = ctx.enter_context(tc.tile_pool(name="f2_ps", bufs=2, space="PSUM"))

        iden = const.tile([128, 128], f32, name="iden")
        nc.sync.dma_start(iden[:], id_d)

        def load_f32r(name, shape, src):
            stg = const.tile(shape, f32, tag=f"{name}_stg", name=f"{name}_stg")
            nc.sync.dma_start(stg[:], src)
            t = const.tile(shape, f32r, tag=name, name=name)
            nc.vector.tensor_copy(t[:], stg[:])
            return t

        kA = load_f32r("kA", [112, 104], kA_d)
        kB = load_f32r("kB", [56, 104], kB_d)
        kC = load_f32r("kC", [112, 52], kC_d)
        w1 = []
        offs = 0
        for b in range(7):
            m = 104 if b < 6 else 52
            w1.append(load_f32r(f"w1_{b}", [m, 128], w1_d[offs:offs + m, :]))
            offs += m
        w2 = load_f32r("w2", [128, 10], w2_d)
        b1 = const.tile([128, 1], f32, name="b1")
        nc.sync.dma_start(b1[:], b1_d)
        b2 = const.tile([10, 1], f32, name="b2")
        nc.sync.dma_start(b2[:], b2_d)

        for it in range(nt):
            # ---- load one batch tile, batch-major [128, 4, 784]
            xbm = xbm_p.tile([128, 4, 784], f32, name="xbm", tag="xbm")
            src = x_d[it * BT:(it + 1) * BT, :].rearrange("(c p) d -> p c d", p=128)
            (nc.sync if it % 2 == 0 else nc.scalar).dma_start(xbm[:], src)

            # ---- transpose to feature-major tiles xfm[t] = x.T rows 112t..112t+111
            xfm = []
            for t in range(7):
                tp = tp_ps.tile([112, BT], f32, name="tp", tag="tp")
                for c in range(4):
                    nc.tensor.transpose(tp[:, c * 128:(c + 1) * 128],
                                        xbm[:, c, 112 * t:112 * t + 112],
                                        iden[:])
                xf = xfm_p.tile([112, BT], f32r, tag="xfm", name=f"xfm{t}")
                if t % 2 == 0:
                    nc.vector.tensor_copy(xf[:], tp[:])
                else:
                    nc.scalar.copy(xf[:], tp[:])
                xfm.append(xf)

            # ---- conv as banded matmuls, relu into h blocks
            hs = []
            for b in range(6):
                cv = cv_ps.tile([104, BT], f32, name="cv", tag="cv")
                nc.tensor.matmul(cv[:], kA[:], xfm[b][:], start=True, stop=False)
                nc.tensor.matmul(cv[:], kB[:], xfm[b + 1][0:56, :],
                                 start=False, stop=True)
                h = h_p.tile([104, BT], f32r, tag="h", name=f"h{b}")
                if b % 2 == 0:
                    nc.vector.tensor_scalar_max(h[:], cv[:], 0.0)
                else:
                    nc.scalar.activation(h[:], cv[:], AF.Relu)
                hs.append(h)
            cv = cv_ps.tile([52, BT], f32, name="cv6", tag="cv")
            nc.tensor.matmul(cv[:], kC[:], xfm[6][:], start=True, stop=True)
            h = h_p.tile([52, BT], f32r, tag="h", name="h6")
            nc.vector.tensor_scalar_max(h[:], cv[:], 0.0)
            hs.append(h)

            # ---- fc1: accumulate 7 chunks, relu + bias
            f1 = f1_ps.tile([128, BT], f32, name="f1", tag="f1")
            for b in range(7):
                nc.tensor.matmul(f1[:], w1[b][:], hs[b][:],
                                 start=(b == 0), stop=(b == 6))
            h1 = h1_p.tile([128, BT], f32r, name="h1", tag="h1")
            nc.scalar.activation(h1[:], f1[:], AF.Relu, bias=b1[:])

            # ---- fc2 + bias
            f2 = f2_ps.tile([10, BT], f32, name="f2", tag="f2")
            nc.tensor.matmul(f2[:], w2[:], h1[:], start=True, stop=True)
            osb = o_p.tile([10, BT], f32, name="osb", tag="osb")
            nc.scalar.activation(osb[:], f2[:], AF.Identity, bias=b2[:])

            # ---- store (feature-major; host transposes)
            nc.sync.dma_start(y_d[:, it * BT:(it + 1) * BT], osb[:])

    nc.compile()
    return nc


def _host_prep(inputs):
    x = np.ascontiguousarray(np.asarray(inputs["x"], dtype=np.float32))
    w = np.asarray(inputs["conv_w"], dtype=np.float32)
    fc1_w = np.asarray(inputs["fc1_w"], dtype=np.float32)
    fc1_b = np.asarray(inputs["fc1_b"], dtype=np.float32)
    fc2_w = np.asarray(inputs["fc2_w"], dtype=np.float32)
    fc2_b = np.asarray(inputs["fc2_b"], dtype=np.float32)

    kA = np.zeros((112, 104), np.float32)
    kB = np.zeros((56, 104), np.float32)
    kC = np.zeros((112, 52), np.float32)
    for oi in range(4):
        for oj in range(26):
            m = oi * 26 + oj
            for di in range(3):
                for dj in range(3):
                    ri, ci = oi + di, oj + dj
                    if ri < 4:
                        kA[ri * 28 + ci, m] = w[di, dj]
                    else:
                        kB[(ri - 4) * 28 + ci, m] = w[di, dj]
    for oi in range(2):
        for oj in range(26):
            m = oi * 26 + oj
            for di in range(3):
                for dj in range(3):
                    kC[(oi + di) * 28 + (oj + dj), m] = w[di, dj]

    consts = {
        "kA": kA,
        "kB": kB,
        "kC": kC,
        "w1": np.ascontiguousarray(fc1_w.T),
        "b1": np.ascontiguousarray(fc1_b.reshape(128, 1)),
        "w2": np.ascontiguousarray(fc2_w.T),
        "b2": np.ascontiguousarray(fc2_b.reshape(10, 1)),
        "iden": np.eye(128, dtype=np.float32),
    }
    in_maps = []
    for c in range(N_CORES):
        m = {"x": x[c * B_CORE:(c + 1) * B_CORE]}
        m.update(consts)
        in_maps.append(m)
    return in_maps


GBT = 2048                  # batch rows per DMA-transpose group (4 tiles)


def _build_module_v4(b_core=B_CORE, n_cores=N_CORES):
    import concourse.bass as bass
    import concourse.tile as tile
    from concourse import bacc, mybir

    f32 = mybir.dt.float32
    f32r = mybir.dt.float32r
    bf16 = mybir.dt.bfloat16
    AF = mybir.ActivationFunctionType
    nt = b_core // BT

    nc = bacc.Bacc("TRN2", target_bir_lowering=False, debug=False,
                   num_devices=n_cores)

    x_d = nc.dram_tensor("x", [b_core, 784], bf16, kind="ExternalInput").ap()
    kA_d = nc.dram_tensor("kA", [112, 104], bf16, kind="ExternalInput").ap()
    kB_d = nc.dram_tensor("kB", [56, 104], bf16, kind="ExternalInput").ap()
    kC_d = nc.dram_tensor("kC", [112, 52], bf16, kind="ExternalInput").ap()
    id_d = nc.dram_tensor("iden", [128, 128], bf16, kind="ExternalInput").ap()
    w1_d = nc.dram_tensor("w1", [676, 128], f32, kind="ExternalInput").ap()
    b1_d = nc.dram_tensor("b1", [128, 1], f32, kind="ExternalInput").ap()
    w2_d = nc.dram_tensor("w2", [128, 10], f32, kind="ExternalInput").ap()
    b2_d = nc.dram_tensor("b2", [10, 1], f32, kind="ExternalInput").ap()
    y_d = nc.dram_tensor("y", [10, b_core], f32, kind="ExternalOutput").ap()

    with tile.TileContext(nc) as tc, ExitStack() as ctx:
        const = ctx.enter_context(tc.tile_pool(name="const", bufs=1))
        xbm_p = ctx.enter_context(tc.tile_pool(name="xbm", bufs=4))
        xfm_p = ctx.enter_context(tc.tile_pool(name="xfm", bufs=21))
        h_p = ctx.enter_context(tc.tile_pool(name="h", bufs=14))
        h1_p = ctx.enter_context(tc.tile_pool(name="h1", bufs=2))
        o_p = ctx.enter_context(tc.tile_pool(name="osb", bufs=1))
        tp_ps = ctx.enter_context(tc.tile_pool(name="tp_ps", bufs=2, space="PSUM"))
        cv_ps = ctx.enter_context(tc.tile_pool(name="cv_ps", bufs=3, space="PSUM"))
        f1_ps = ctx.enter_context(tc.tile_pool(name="f1_ps", bufs=2, space="PSUM"))
        f2_ps = ctx.enter_context(tc.tile_pool(name="f2_ps", bufs=1, space="PSUM"))

        iden = const.tile([128, 128], bf16, name="iden")
        nc.sync.dma_start(iden[:], id_d)
        kA = const.tile([112, 104], bf16, name="kA")
        nc.sync.dma_start(kA[:], kA_d)
        kB = const.tile([56, 104], bf16, name="kB")
        nc.sync.dma_start(kB[:], kB_d)
        kC = const.tile([112, 52], bf16, name="kC")
        nc.sync.dma_start(kC[:], kC_d)

        def load_f32r(name, shape, src):
            stg = const.tile(shape, f32, tag=f"{name}_stg", name=f"{name}_stg")
            nc.sync.dma_start(stg[:], src)
            t = const.tile(shape, f32r, tag=name, name=name)
            nc.vector.tensor_copy(t[:], stg[:])
            return t

        w1 = []
        offs = 0
        for b in range(7):
            m = 104 if b < 6 else 52
            w1.append(load_f32r(f"w1_{b}", [m, 128], w1_d[offs:offs + m, :]))
            offs += m
        w2 = load_f32r("w2", [128, 10], w2_d)
        b1 = const.tile([128, 1], f32, name="b1")
        nc.sync.dma_start(b1[:], b1_d)
        b2 = const.tile([10, 1], f32, name="b2")
        nc.sync.dma_start(b2[:], b2_d)

        y_sb = o_p.tile([10, b_core], f32, name="y_sb")

        # Phase-separated pairs: all transposes for two batch tiles, then one
        # dense block of 42 real matmuls so the PE clock-gate releases.
        def tp_group(xbm, xfm, t, eng):
            tp = tp_ps.tile([112, BT], bf16, name="tp", tag="tp")
            for c in range(4):
                nc.tensor.transpose(tp[:, c * 128:(c + 1) * 128],
                                    xbm[:, c, 112 * t:112 * t + 112],
                                    iden[:])
            xf = xfm_p.tile([112, BT], bf16, tag="xfm", name=f"xfm{t}")
            if eng == 0:
                nc.vector.tensor_copy(xf[:], tp[:])
            else:
                nc.scalar.copy(xf[:], tp[:])
            xfm.append(xf)

        def conv_block(xfm, hs, b, eng):
            if b < 6:
                cv = cv_ps.tile([104, BT], f32, name="cv", tag="cv")
                nc.tensor.matmul(cv[:], kA[:], xfm[b][:], start=True, stop=False)
                nc.tensor.matmul(cv[:], kB[:], xfm[b + 1][0:56, :],
                                 start=False, stop=True)
                h = h_p.tile([104, BT], f32r, tag="h", name=f"h{b}")
            else:
                cv = cv_ps.tile([52, BT], f32, name="cv6", tag="cv")
                nc.tensor.matmul(cv[:], kC[:], xfm[6][:], start=True, stop=True)
                h = h_p.tile([52, BT], f32r, tag="h", name="h6")
            if eng == 0:
                nc.vector.tensor_scalar_max(h[:], cv[:], 0.0)
            else:
                nc.scalar.activation(h[:], cv[:], AF.Relu)
            hs.append(h)

        for it0 in range(0, nt, 2):
            its = [it0, it0 + 1]
            xfms = []
            hss = []
            for k, it in enumerate(its):
                xbm = xbm_p.tile([128, 4, 784], bf16, name="xbm", tag="xbm")
                src = x_d[it * BT:(it + 1) * BT, :].rearrange(
                    "(c p) d -> p c d", p=128)
                (nc.sync if it % 2 == 0 else nc.scalar).dma_start(xbm[:], src)
                xfm = []
                for t in range(7):
                    tp_group(xbm, xfm, t, (t + k) % 2)
                xfms.append(xfm)
                hss.append([])

            for k in range(2):
                for b in range(7):
                    conv_block(xfms[k], hss[k], b, (b + k) % 2)

            f1s = []
            for k, it in enumerate(its):
                f1s.append(f1_ps.tile([128, BT], f32, name=f"f1_{k}", tag="f1"))
            for b in range(7):
                for k in range(2):
                    nc.tensor.matmul(f1s[k][:], w1[b][:], hss[k][b][:],
                                     start=(b == 0), stop=(b == 6))
            for k, it in enumerate(its):
                h1 = h1_p.tile([128, BT], f32r, name=f"h1_{k}", tag="h1")
                nc.scalar.activation(h1[:], f1s[k][:], AF.Relu, bias=b1[:])
                f2 = f2_ps.tile([10, BT], f32, name=f"f2_{k}", tag="f2")
                nc.tensor.matmul(f2[:], w2[:], h1[:], start=True, stop=True)
                nc.scalar.activation(y_sb[:, it * BT:(it + 1) * BT], f2[:],
                                     AF.Identity, bias=b2[:])

        # ---- single store at the very end (feature-major; host transposes)
        nc.sync.dma_start(y_d, y_sb[:])

    nc.compile()
    return nc


def _host_prep_v4(inputs):
    import ml_dtypes
    bf = ml_dtypes.bfloat16
    x = np.asarray(inputs["x"], dtype=np.float32)
    w = np.asarray(inputs["conv_w"], dtype=np.float32)
    fc1_w = np.asarray(inputs["fc1_w"], dtype=np.float32)
    fc1_b = np.asarray(inputs["fc1_b"], dtype=np.float32)
    fc2_w = np.asarray(inputs["fc2_w"], dtype=np.float32)
    fc2_b = np.asarray(inputs["fc2_b"], dtype=np.float32)

    xp = np.ascontiguousarray(x.astype(bf))

    kA = np.zeros((112, 104), np.float32)
    kB = np.zeros((56, 104), np.float32)
    kC = np.zeros((112, 52), np.float32)
    for oi in range(4):
        for oj in range(26):
            m = oi * 26 + oj
            for di in range(3):
                for dj in range(3):
                    ri, ci = oi + di, oj + dj
                    if ri < 4:
                        kA[ri * 28 + ci, m] = w[di, dj]
                    else:
                        kB[(ri - 4) * 28 + ci, m] = w[di, dj]
    for oi in range(2):
        for oj in range(26):
            m = oi * 26 + oj
            for di in range(3):
                for dj in range(3):
                    kC[(oi + di) * 28 + (oj + dj), m] = w[di, dj]

    consts = {
        "kA": kA.astype(bf),
        "kB": kB.astype(bf),
        "kC": kC.astype(bf),
        "iden": np.eye(128, dtype=np.float32).astype(bf),
        "w1": np.ascontiguousarray(fc1_w.T),
        "b1": np.ascontiguousarray(fc1_b.reshape(128, 1)),
        "w2": np.ascontiguousarray(fc2_w.T),
        "b2": np.ascontiguousarray(fc2_b.reshape(10, 1)),
    }
    in_maps = []
    for c in range(N_CORES):
        m = {"x": xp[c * B_CORE:(c + 1) * B_CORE]}
        m.update(consts)
        in_maps.append(m)
    return in_maps



def _build_module_v6(b_core=B_CORE, n_cores=N_CORES):
    import concourse.bass as bass
    import concourse.tile as tile
    from concourse import bacc, mybir

    f32 = mybir.dt.float32
    f32r = mybir.dt.float32r
    bf16 = mybir.dt.bfloat16
    AF = mybir.ActivationFunctionType
    nt = b_core // BT

    nc = bacc.Bacc("TRN2", target_bir_lowering=False, debug=False,
                   num_devices=n_cores)

    # x arrives feature-major from the host: [7 row-groups, 112 pixels, batch]
    x_d = nc.dram_tensor("x", [7, 112, b_core], bf16, kind="ExternalInput").ap()
    kA_d = nc.dram_tensor("kA", [112, 104], bf16, kind="ExternalInput").ap()
    kB_d = nc.dram_tensor("kB", [56, 104], bf16, kind="ExternalInput").ap()
    kC_d = nc.dram_tensor("kC", [112, 52], bf16, kind="ExternalInput").ap()
    w1_d = nc.dram_tensor("w1", [676, 128], bf16, kind="ExternalInput").ap()
    b1_d = nc.dram_tensor("b1", [128, 1], f32, kind="ExternalInput").ap()
    w2_d = nc.dram_tensor("w2", [128, 10], bf16, kind="ExternalInput").ap()
    b2_d = nc.dram_tensor("b2", [10, 1], f32, kind="ExternalInput").ap()
    y_d = nc.dram_tensor("y", [10, b_core], f32, kind="ExternalOutput").ap()

    with tile.TileContext(nc) as tc, ExitStack() as ctx:
        const = ctx.enter_context(tc.tile_pool(name="const", bufs=1))
        xfm_p = ctx.enter_context(tc.tile_pool(name="xfm", bufs=3))
        h_p = ctx.enter_context(tc.tile_pool(name="h", bufs=14))
        h1_p = ctx.enter_context(tc.tile_pool(name="h1", bufs=2))
        o_p = ctx.enter_context(tc.tile_pool(name="osb", bufs=1))
        cv_ps = ctx.enter_context(tc.tile_pool(name="cv_ps", bufs=4, space="PSUM"))
        f1_ps = ctx.enter_context(tc.tile_pool(name="f1_ps", bufs=2, space="PSUM"))
        f2_ps = ctx.enter_context(tc.tile_pool(name="f2_ps", bufs=2, space="PSUM"))

        kA = const.tile([112, 104], bf16, name="kA")
        nc.sync.dma_start(kA[:], kA_d)
        kB = const.tile([56, 104], bf16, name="kB")
        nc.sync.dma_start(kB[:], kB_d)
        kC = const.tile([112, 52], bf16, name="kC")
        nc.sync.dma_start(kC[:], kC_d)

        w1 = []
        offs = 0
        for b in range(7):
            m = 104 if b < 6 else 52
            t = const.tile([m, 128], bf16, tag=f"w1_{b}", name=f"w1_{b}")
            nc.sync.dma_start(t[:], w1_d[offs:offs + m, :])
            w1.append(t)
            offs += m
        w2 = const.tile([128, 10], bf16, name="w2")
        nc.sync.dma_start(w2[:], w2_d)
        b1 = const.tile([128, 1], f32, name="b1")
        nc.sync.dma_start(b1[:], b1_d)
        b2 = const.tile([10, 1], f32, name="b2")
        nc.sync.dma_start(b2[:], b2_d)

        y_sb = o_p.tile([10, b_core], f32, name="y_sb")

        # Two batch-tiles are processed as interleaved instruction streams:
        # consecutive PE matmuls belong to independent tiles (different PSUM
        # banks, independent deps) so fill/drain phases overlap.
        def emit_pair(its):
            xfms, hss, f1s, h1s = [], [], [], []
            for k, it in enumerate(its):
                xfm = xfm_p.tile([112, 7, BT], bf16, name="xfm", tag="xfm")
                src = x_d[:, :, it * BT:(it + 1) * BT].rearrange("g p b -> p g b")
                (nc.sync if it % 2 == 0 else nc.scalar).dma_start(xfm[:], src)
                xfms.append(xfm)
                hss.append([])

            for b in range(7):
                cvs = []
                if b < 6:
                    for k in range(len(its)):
                        cv = cv_ps.tile([104, BT], f32, name="cv", tag="cv")
                        nc.tensor.matmul(cv[:], kA[:], xfms[k][:, b, :],
                                         start=True, stop=False)
                        cvs.append(cv)
                    for k in range(len(its)):
                        nc.tensor.matmul(cvs[k][:], kB[:],
                                         xfms[k][0:56, b + 1, :],
                                         start=False, stop=True)
                else:
                    for k in range(len(its)):
                        cv = cv_ps.tile([52, BT], f32, name="cv6", tag="cv")
                        nc.tensor.matmul(cv[:], kC[:], xfms[k][:, 6, :],
                                         start=True, stop=True)
                        cvs.append(cv)
                for k in range(len(its)):
                    h = h_p.tile([104 if b < 6 else 52, BT], bf16, tag="h",
                                 name=f"h{b}_{k}")
                    if (b + k) % 2 == 0:
                        nc.vector.tensor_scalar_max(h[:], cvs[k][:], 0.0)
                    else:
                        nc.scalar.activation(h[:], cvs[k][:], AF.Relu)
                    hss[k].append(h)

            for k, it in enumerate(its):
                f1s.append(f1_ps.tile([128, BT], f32, name=f"f1_{k}", tag="f1"))
            for b in range(7):
                for k in range(len(its)):
                    nc.tensor.matmul(f1s[k][:], w1[b][:], hss[k][b][:],
                                     start=(b == 0), stop=(b == 6))
            for k, it in enumerate(its):
                h1 = h1_p.tile([128, BT], bf16, name=f"h1_{k}", tag="h1")
                nc.scalar.activation(h1[:], f1s[k][:], AF.Relu, bias=b1[:])
                h1s.append(h1)
            for k, it in enumerate(its):
                f2 = f2_ps.tile([10, BT], f32, name=f"f2_{k}", tag="f2")
                nc.tensor.matmul(f2[:], w2[:], h1s[k][:], start=True, stop=True)
                nc.scalar.activation(y_sb[:, it * BT:(it + 1) * BT], f2[:],
                                     AF.Identity, bias=b2[:])

        for it2 in range(0, nt, 2):
            emit_pair([it2, it2 + 1])

        # ---- single store at the very end (feature-major; host transposes)
        nc.sync.dma_start(y_d, y_sb[:])

    nc.compile()
    return nc


def _host_prep_v6(inputs):
    import ml_dtypes
    bf = ml_dtypes.bfloat16
    x = np.asarray(inputs["x"], dtype=np.float32)
    w = np.asarray(inputs["conv_w"], dtype=np.float32)
    fc1_w = np.asarray(inputs["fc1_w"], dtype=np.float32)
    fc1_b = np.asarray(inputs["fc1_b"], dtype=np.float32)
    fc2_w = np.asarray(inputs["fc2_w"], dtype=np.float32)
    fc2_b = np.asarray(inputs["fc2_b"], dtype=np.float32)

    B = x.shape[0]
    # feature-major: [7 row-groups, 112 pixels, B]
    xT = np.ascontiguousarray(x.astype(bf).reshape(B, 7, 112).transpose(1, 2, 0))

    kA = np.zeros((112, 104), np.float32)
    kB = np.zeros((56, 104), np.float32)
    kC = np.zeros((112, 52), np.float32)
    for oi in range(4):
        for oj in range(26):
            m = oi * 26 + oj
            for di in range(3):
                for dj in range(3):
                    ri, ci = oi + di, oj + dj
                    if ri < 4:
                        kA[ri * 28 + ci, m] = w[di, dj]
                    else:
                        kB[(ri - 4) * 28 + ci, m] = w[di, dj]
    for oi in range(2):
        for oj in range(26):
            m = oi * 26 + oj
            for di in range(3):
                for dj in range(3):
                    kC[(oi + di) * 28 + (oj + dj), m] = w[di, dj]

    consts = {
        "kA": kA.astype(bf),
        "kB": kB.astype(bf),
        "kC": kC.astype(bf),
        "w1": np.ascontiguousarray(fc1_w.T.astype(bf)),
        "b1": np.ascontiguousarray(fc1_b.reshape(128, 1)),
        "w2": np.ascontiguousarray(fc2_w.T.astype(bf)),
        "b2": np.ascontiguousarray(fc2_b.reshape(10, 1)),
    }
    in_maps = []
    for c in range(N_CORES):
        m = {"x": np.ascontiguousarray(xT[:, :, c * B_CORE:(c + 1) * B_CORE])}
        m.update(consts)
        in_maps.append(m)
    return in_maps


def _build_module_v7(b_core=B_CORE, n_cores=N_CORES):
    """Feature-major end-to-end, zero PE transposes, 19 matmuls per 512-tile.

    x arrives host-transposed and host-tiled: [nt, 112, 7, BT] so each batch
    tile is one contiguous 0.8 MB DMA (112 partitions x 7 KB descriptors).
    Conv = 6 output chunks of <=116, each 2 accumulating matmuls against the
    two x row-groups its band window spans. fc1 = 6 chunks (matching the conv
    chunking), fc2 = 1. All bf16 weights/activations, fp32 PSUM.
    The PE stream is pure back-to-back real matmuls so the HAM clock gate
    warms once and stays at K=8/8 (the v4 baseline spent 63% of its span
    throttled at half clock because transposes don't count as PE activity).
    """
    import concourse.bass as bass
    import concourse.tile as tile
    from concourse import bacc, mybir

    f32 = mybir.dt.float32
    bf16 = mybir.dt.bfloat16
    AF = mybir.ActivationFunctionType
    nt = b_core // BT

    nc = bacc.Bacc("TRN2", target_bir_lowering=False, debug=False,
                   num_devices=n_cores)

    x_d = nc.dram_tensor("x", [nt, 112, 7, BT], bf16, kind="ExternalInput").ap()
    ka_d = nc.dram_tensor("ka", [112, 6, 116], bf16, kind="ExternalInput").ap()
    kb_d = nc.dram_tensor("kb", [112, 6, 116], bf16, kind="ExternalInput").ap()
    w1_d = nc.dram_tensor("w1", [116, 6, 128], bf16, kind="ExternalInput").ap()
    w2_d = nc.dram_tensor("w2", [128, 10], bf16, kind="ExternalInput").ap()
    b1_d = nc.dram_tensor("b1", [128, 1], f32, kind="ExternalInput").ap()
    y_d = nc.dram_tensor("y", [10, b_core], f32, kind="ExternalOutput").ap()

    with tile.TileContext(nc) as tc, ExitStack() as ctx:
        const = ctx.enter_context(tc.tile_pool(name="const", bufs=1))
        xfm_p = ctx.enter_context(tc.tile_pool(name="xfm", bufs=3))
        h_p = ctx.enter_context(tc.tile_pool(name="h", bufs=14))
        h1_p = ctx.enter_context(tc.tile_pool(name="h1", bufs=2))
        o_p = ctx.enter_context(tc.tile_pool(name="osb", bufs=1))
        cv_ps = ctx.enter_context(tc.tile_pool(name="cv_ps", bufs=4, space="PSUM"))
        f1_ps = ctx.enter_context(tc.tile_pool(name="f1_ps", bufs=2, space="PSUM"))
        f2_ps = ctx.enter_context(tc.tile_pool(name="f2_ps", bufs=2, space="PSUM"))

        xts = [None] * nt

        def load_x(it):
            t = xfm_p.tile([112, 7, BT], bf16, name="xfm", tag="xfm")
            (nc.sync if it % 2 == 0 else nc.gpsimd).dma_start(t[:], x_d[it])
            xts[it] = t

        load_x(0)
        ka = const.tile([112, 6, 116], bf16, name="ka")
        nc.sync.dma_start(ka[:], ka_d)
        kb = const.tile([112, 6, 116], bf16, name="kb")
        nc.sync.dma_start(kb[:], kb_d)
        w1 = const.tile([116, 6, 128], bf16, name="w1")
        nc.sync.dma_start(w1[:], w1_d)
        w2 = const.tile([128, 10], bf16, name="w2")
        nc.sync.dma_start(w2[:], w2_d)
        b1 = const.tile([128, 1], f32, name="b1")
        nc.sync.dma_start(b1[:], b1_d)
        load_x(1)

        y_sb = o_p.tile([10, b_core], f32, name="y_sb")

        def emit_conv(it):
            xfm = xts[it]
            hs = []
            for c in range(6):
                cv = cv_ps.tile([116, BT], f32, name="cv", tag="cv")
                nc.tensor.matmul(cv[:], ka[:, c, :], xfm[:, c, :],
                                 start=True, stop=False)
                nc.tensor.matmul(cv[:], kb[:, c, :], xfm[:, c + 1, :],
                                 start=False, stop=True)
                h = h_p.tile([116, BT], bf16, tag="h", name=f"h{c}")
                if c % 2 == 0:
                    nc.vector.tensor_scalar_max(h[:], cv[:], 0.0)
                else:
                    nc.scalar.activation(h[:], cv[:], AF.Relu)
                hs.append(h)
            return hs

        def emit_fc(it, hs):
            f1 = f1_ps.tile([128, BT], f32, name="f1", tag="f1")
            for c in range(6):
                nc.tensor.matmul(f1[:], w1[:, c, :], hs[c][:],
                                 start=(c == 0), stop=(c == 5))
            h1 = h1_p.tile([128, BT], bf16, name="h1", tag="h1")
            nc.scalar.activation(h1[:], f1[:], AF.Relu, bias=b1[:])
            f2 = f2_ps.tile([10, BT], f32, name="f2", tag="f2")
            nc.tensor.matmul(f2[:], w2[:], h1[:], start=True, stop=True)
            nc.vector.tensor_copy(y_sb[:, it * BT:(it + 1) * BT], f2[:])

        prev_hs = None
        for it in range(nt):
            if it + 2 < nt:
                load_x(it + 2)
            hs = emit_conv(it)
            if prev_hs is not None:
                emit_fc(it - 1, prev_hs)
            prev_hs = hs
        emit_fc(nt - 1, prev_hs)

        nc.sync.dma_start(y_d, y_sb[:])

    nc.compile()
    return nc


def _build_module_v8(b_core=B_CORE, n_cores=N_CORES):
    """v7 + pipelined prologue/epilogue and bank-conflict-free PE stream.

    - conv/fc1 weights load on the scalar HWDGE queue, concurrent with x
      tiles on the sync/gpsimd queues (v7 serialized them: first MM at 17.6us).
    - 8 dummy matmuls on a zeroed tile warm the HAM clock gate during the
      DMA prologue so real matmuls start at 2.4 GHz.
    - 2-deep pipeline: iteration i emits conv(i), fc1(i-1), fc2(i-2)
      interleaved [f2, cvA0, f1_0, cvB0, cvA1, f1_1, cvB1, ...] so no two
      adjacent PE instructions touch the same PSUM bank (consecutive
      accumulates into one bank stall the array by the drain latency).
    - y flushed to DRAM every 4 tiles instead of once at the end.
    """
    import concourse.bass as bass
    import concourse.tile as tile
    from concourse import bacc, mybir

    f32 = mybir.dt.float32
    bf16 = mybir.dt.bfloat16
    AF = mybir.ActivationFunctionType
    nt = b_core // BT

    nc = bacc.Bacc("TRN2", target_bir_lowering=False, debug=False,
                   num_devices=n_cores)

    x_d = nc.dram_tensor("x", [nt, 112, 7, BT], bf16, kind="ExternalInput").ap()
    ka_d = nc.dram_tensor("ka", [112, 6, 116], bf16, kind="ExternalInput").ap()
    kb_d = nc.dram_tensor("kb", [112, 6, 116], bf16, kind="ExternalInput").ap()
    w1_d = nc.dram_tensor("w1", [116, 6, 128], bf16, kind="ExternalInput").ap()
    w2_d = nc.dram_tensor("w2", [128, 10], bf16, kind="ExternalInput").ap()
    b1_d = nc.dram_tensor("b1", [128, 1], f32, kind="ExternalInput").ap()
    y_d = nc.dram_tensor("y", [10, b_core], f32, kind="ExternalOutput").ap()

    with tile.TileContext(nc) as tc, ExitStack() as ctx:
        const = ctx.enter_context(tc.tile_pool(name="const", bufs=1))
        xfm_p = ctx.enter_context(tc.tile_pool(name="xfm", bufs=3))
        h_p = ctx.enter_context(tc.tile_pool(name="h", bufs=14))
        h1_p = ctx.enter_context(tc.tile_pool(name="h1", bufs=3))
        o_p = ctx.enter_context(tc.tile_pool(name="osb", bufs=1))
        cv_ps = ctx.enter_context(tc.tile_pool(name="cv_ps", bufs=4, space="PSUM"))
        f1_ps = ctx.enter_context(tc.tile_pool(name="f1_ps", bufs=2, space="PSUM"))
        f2_ps = ctx.enter_context(tc.tile_pool(name="f2_ps", bufs=2, space="PSUM"))

        # PE warm-up source: a zeroed SBUF tile (values irrelevant).
        dummy = const.tile([128, BT], bf16, name="dummy")
        nc.scalar.memzero(dummy[:])

        ka = const.tile([112, 6, 116], bf16, name="ka")
        nc.scalar.dma_start(ka[:], ka_d)
        kb = const.tile([112, 6, 116], bf16, name="kb")
        nc.scalar.dma_start(kb[:], kb_d)
        w1 = const.tile([116, 6, 128], bf16, name="w1")
        nc.scalar.dma_start(w1[:], w1_d)
        w2 = const.tile([128, 10], bf16, name="w2")
        nc.scalar.dma_start(w2[:], w2_d)
        b1 = const.tile([128, 1], f32, name="b1")
        nc.scalar.dma_start(b1[:], b1_d)

        xts = [None] * nt

        def load_x(it):
            t = xfm_p.tile([112, 7, BT], bf16, name="xfm", tag="xfm")
            (nc.sync if it % 2 == 0 else nc.gpsimd).dma_start(t[:], x_d[it])
            xts[it] = t

        load_x(0)
        load_x(1)

        for _ in range(8):
            wm = f2_ps.tile([10, BT], f32, name="warm", tag="f2")
            nc.tensor.matmul(wm[:], dummy[:, 0:10], dummy[:],
                             start=True, stop=True)

        y_sb = o_p.tile([10, b_core], f32, name="y_sb")

        hs_hist = [None] * nt
        h1_hist = [None] * nt

        for i in range(nt + 2):
            conv_it = i if i < nt else None
            fc1_it = i - 1 if 0 <= i - 1 < nt else None
            fc2_it = i - 2 if 0 <= i - 2 else None

            if conv_it is not None and conv_it + 2 < nt:
                load_x(conv_it + 2)

            # fc2 of tile i-2 first (its h1 has been ready for a while)
            if fc2_it is not None:
                f2 = f2_ps.tile([10, BT], f32, name="f2", tag="f2")
                nc.tensor.matmul(f2[:], w2[:], h1_hist[fc2_it][:],
                                 start=True, stop=True)
                nc.vector.tensor_copy(y_sb[:, fc2_it * BT:(fc2_it + 1) * BT],
                                      f2[:])
                if fc2_it % 4 == 3:
                    lo = (fc2_it - 3) * BT
                    hi = (fc2_it + 1) * BT
                    nc.sync.dma_start(y_d[:, lo:hi], y_sb[:, lo:hi])

            if fc1_it is not None:
                f1 = f1_ps.tile([128, BT], f32, name="f1", tag="f1")
                prev_hs = hs_hist[fc1_it]

            xfm = xts[conv_it] if conv_it is not None else None
            hs = []
            for c in range(6):
                cv = None
                if xfm is not None:
                    cv = cv_ps.tile([116, BT], f32, name="cv", tag="cv")
                    nc.tensor.matmul(cv[:], ka[:, c, :], xfm[:, c, :],
                                     start=True, stop=False)
                if fc1_it is not None:
                    nc.tensor.matmul(f1[:], w1[:, c, :], prev_hs[c][:],
                                     start=(c == 0), stop=(c == 5))
                if xfm is not None:
                    nc.tensor.matmul(cv[:], kb[:, c, :], xfm[:, c + 1, :],
                                     start=False, stop=True)
                    h = h_p.tile([116, BT], bf16, tag="h", name=f"h{c}")
                    if c % 2 == 0:
                        nc.vector.tensor_scalar_max(h[:], cv[:], 0.0)
                    else:
                        nc.scalar.activation(h[:], cv[:], AF.Relu)
                    hs.append(h)
            if conv_it is not None:
                hs_hist[conv_it] = hs

            if fc1_it is not None:
                h1 = h1_p.tile([128, BT], bf16, name="h1", tag="h1")
                nc.scalar.activation(h1[:], f1[:], AF.Relu, bias=b1[:])
                h1_hist[fc1_it] = h1

    nc.compile()
    return nc


_V7_BOUNDS = [0, 112, 224, 336, 448, 560, 676]


def _host_prep_v7(inputs):
    import ml_dtypes
    bf = ml_dtypes.bfloat16
    x = np.asarray(inputs["x"], dtype=np.float32)
    w = np.asarray(inputs["conv_w"], dtype=np.float32)
    fc1_w = np.asarray(inputs["fc1_w"], dtype=np.float32)
    fc1_b = np.asarray(inputs["fc1_b"], dtype=np.float32)
    fc2_w = np.asarray(inputs["fc2_w"], dtype=np.float32)

    B = x.shape[0]
    # [B, 784] -> per-tile feature-major [B/BT, 112, 7, BT]
    xr = x.astype(bf).reshape(B // BT, BT, 7, 112)
    xt = np.ascontiguousarray(xr.transpose(0, 3, 2, 1))

    ka = np.zeros((112, 6, 116), np.float32)
    kb = np.zeros((112, 6, 116), np.float32)
    for c in range(6):
        o0, o1 = _V7_BOUNDS[c], _V7_BOUNDS[c + 1]
        for m in range(o1 - o0):
            oi, oj = divmod(o0 + m, 26)
            for di in range(3):
                for dj in range(3):
                    g, p = divmod(28 * (oi + di) + (oj + dj), 112)
                    if g == c:
                        ka[p, c, m] = w[di, dj]
                    else:
                        assert g == c + 1, (c, o0 + m, g)
                        kb[p, c, m] = w[di, dj]

    w1T = fc1_w.T  # [676, 128]
    w1u = np.zeros((116, 6, 128), np.float32)
    for c in range(6):
        o0, o1 = _V7_BOUNDS[c], _V7_BOUNDS[c + 1]
        w1u[0:o1 - o0, c, :] = w1T[o0:o1, :]

    consts = {
        "ka": ka.astype(bf),
        "kb": kb.astype(bf),
        "w1": w1u.astype(bf),
        "w2": np.ascontiguousarray(fc2_w.T.astype(bf)),
        "b1": np.ascontiguousarray(fc1_b.reshape(128, 1)),
    }
    in_maps = []
    for c in range(N_CORES):
        nt = B_CORE // BT
        m = {"x": np.ascontiguousarray(xt[c * nt:(c + 1) * nt])}
        m.update(consts)
        in_maps.append(m)
    return in_maps


VERSION = 8


def run(inputs, trace=False, tmpdir=None, version=None):
    from concourse.bass_utils import run_bass_kernel_spmd

    version = VERSION if version is None else version
    key = f"nc{version}"
    builders = {8: _build_module_v8, 7: _build_module_v7, 6: _build_module_v6,
                4: _build_module_v4, 2: _build_module}
    preps = {8: _host_prep_v7, 7: _host_prep_v7, 6: _host_prep_v6,
             4: _host_prep_v4, 2: _host_prep}
    if key not in _cache:
        _cache[key] = builders[version]()
    nc = _cache[key]
    in_maps = preps[version](inputs)
    res = run_bass_kernel_spmd(nc, in_maps, list(range(N_CORES)), trace=trace,
                               tmpdir=tmpdir)
    out = np.concatenate([np.ascontiguousarray(r["y"].T) for r in res.results], axis=0)
    if version >= 7:
        # fc2 bias is folded in on the host for v7+.
        out = out + np.asarray(inputs["fc2_b"], dtype=np.float32)[None, :]
    return out, res


def kernel(**inputs) -> np.ndarray:
    out, _ = run(inputs, trace=False)
    return out



# revision 7
# speedup vs baseline: 1.0233x; 1.0233x over previous
"""Trainium2 Bass kernel for DigitConvolutionalModel.

Pipeline (per core, pure data-parallel over batch):
  x [8192, 784] --DMA--> SBUF batch-major --PE transpose--> feature-major tiles
  conv 3x3 as banded block-matmuls on PE -> relu -> fc1 (matmul) -> relu
  -> fc2 (matmul) + bias -> DMA out.

All activations live feature-major ([features, batch]) so the PE can contract
over the partition dim. The 3x3 conv is expressed as 13 small banded matmuls
per 512-batch tile using three constant band matrices built on the host from
conv_w (shift-invariant across 4-image-row blocks).
"""

import numpy as np
from contextlib import ExitStack

N_CORES = 8
B_FULL = 65536
B_CORE = B_FULL // N_CORES  # 8192
BT = 512                    # batch tile (matmul moving free dim)
NT = B_CORE // BT           # 16

_cache = {}


def _build_module(b_core=B_CORE, n_cores=N_CORES):
    import concourse.bass as bass
    import concourse.tile as tile
    from concourse import bacc, mybir

    f32 = mybir.dt.float32
    f32r = mybir.dt.float32r
    AF = mybir.ActivationFunctionType
    nt = b_core // BT

    nc = bacc.Bacc("TRN2", target_bir_lowering=False, debug=False,
                   num_devices=n_cores)

    x_d = nc.dram_tensor("x", [b_core, 784], f32, kind="ExternalInput").ap()
    kA_d = nc.dram_tensor("kA", [112, 104], f32, kind="ExternalInput").ap()
    kB_d = nc.dram_tensor("kB", [56, 104], f32, kind="ExternalInput").ap()
    kC_d = nc.dram_tensor("kC", [112, 52], f32, kind="ExternalInput").ap()
    w1_d = nc.dram_tensor("w1", [676, 128], f32, kind="ExternalInput").ap()
    b1_d = nc.dram_tensor("b1", [128, 1], f32, kind="ExternalInput").ap()
    w2_d = nc.dram_tensor("w2", [128, 10], f32, kind="ExternalInput").ap()
    b2_d = nc.dram_tensor("b2", [10, 1], f32, kind="ExternalInput").ap()
    id_d = nc.dram_tensor("iden", [128, 128], f32, kind="ExternalInput").ap()
    y_d = nc.dram_tensor("y", [10, b_core], f32, kind="ExternalOutput").ap()

    with tile.TileContext(nc) as tc, ExitStack() as ctx:
        const = ctx.enter_context(tc.tile_pool(name="const", bufs=1))
        xbm_p = ctx.enter_context(tc.tile_pool(name="xbm", bufs=4))
        xfm_p = ctx.enter_context(tc.tile_pool(name="xfm", bufs=21))
        h_p = ctx.enter_context(tc.tile_pool(name="h", bufs=14))
        h1_p = ctx.enter_context(tc.tile_pool(name="h1", bufs=2))
        o_p = ctx.enter_context(tc.tile_pool(name="osb", bufs=2))
        tp_ps = ctx.enter_context(tc.tile_pool(name="tp_ps", bufs=2, space="PSUM"))
        cv_ps = ctx.enter_context(tc.tile_pool(name="cv_ps", bufs=2, space="PSUM"))
        f1_ps = ctx.enter_context(tc.tile_pool(name="f1_ps", bufs=2, space="PSUM"))
        f2_ps = ctx.enter_context(tc.tile_pool(name="f2_ps", bufs=2, space="PSUM"))

        iden = const.tile([128, 128], f32, name="iden")
        nc.sync.dma_start(iden[:], id_d)

        def load_f32r(name, shape, src):
            stg = const.tile(shape, f32, tag=f"{name}_stg", name=f"{name}_stg")
            nc.sync.dma_start(stg[:], src)
            t = const.tile(shape, f32r, tag=name, name=name)
            nc.vector.tensor_copy(t[:], stg[:])
            return t

        kA = load_f32r("kA", [112, 104], kA_d)
        kB = load_f32r("kB", [56, 104], kB_d)
        kC = load_f32r("kC", [112, 52], kC_d)
        w1 = []
        offs = 0
        for b in range(7):
            m = 104 if b < 6 else 52
            w1.append(load_f32r(f"w1_{b}", [m, 128], w1_d[offs:offs + m, :]))
            offs += m
        w2 = load_f32r("w2", [128, 10], w2_d)
        b1 = const.tile([128, 1], f32, name="b1")
        nc.sync.dma_start(b1[:], b1_d)
        b2 = const.tile([10, 1], f32, name="b2")
        nc.sync.dma_start(b2[:], b2_d)

        for it in range(nt):
            # ---- load one batch tile, batch-major [128, 4, 784]
            xbm = xbm_p.tile([128, 4, 784], f32, name="xbm", tag="xbm")
            src = x_d[it * BT:(it + 1) * BT, :].rearrange("(c p) d -> p c d", p=128)
            (nc.sync if it % 2 == 0 else nc.scalar).dma_start(xbm[:], src)

            # ---- transpose to feature-major tiles xfm[t] = x.T rows 112t..112t+111
            xfm = []
            for t in range(7):
                tp = tp_ps.tile([112, BT], f32, name="tp", tag="tp")
                for c in range(4):
                    nc.tensor.transpose(tp[:, c * 128:(c + 1) * 128],
                                        xbm[:, c, 112 * t:112 * t + 112],
                                        iden[:])
                xf = xfm_p.tile([112, BT], f32r, tag="xfm", name=f"xfm{t}")
                if t % 2 == 0:
                    nc.vector.tensor_copy(xf[:], tp[:])
                else:
                    nc.scalar.copy(xf[:], tp[:])
                xfm.append(xf)

            # ---- conv as banded matmuls, relu into h blocks
            hs = []
            for b in range(6):
                cv = cv_ps.tile([104, BT], f32, name="cv", tag="cv")
                nc.tensor.matmul(cv[:], kA[:], xfm[b][:], start=True, stop=False)
                nc.tensor.matmul(cv[:], kB[:], xfm[b + 1][0:56, :],
                                 start=False, stop=True)
                h = h_p.tile([104, BT], f32r, tag="h", name=f"h{b}")
                if b % 2 == 0:
                    nc.vector.tensor_scalar_max(h[:], cv[:], 0.0)
                else:
                    nc.scalar.activation(h[:], cv[:], AF.Relu)
                hs.append(h)
            cv = cv_ps.tile([52, BT], f32, name="cv6", tag="cv")
            nc.tensor.matmul(cv[:], kC[:], xfm[6][:], start=True, stop=True)
            h = h_p.tile([52, BT], f32r, tag="h", name="h6")
            nc.vector.tensor_scalar_max(h[:], cv[:], 0.0)
            hs.append(h)

            # ---- fc1: accumulate 7 chunks, relu + bias
            f1 = f1_ps.tile([128, BT], f32, name="f1", tag="f1")
            for b in range(7):
                nc.tensor.matmul(f1[:], w1[b][:], hs[b][:],
                                 start=(b == 0), stop=(b == 6))
            h1 = h1_p.tile([128, BT], f32r, name="h1", tag="h1")
            nc.scalar.activation(h1[:], f1[:], AF.Relu, bias=b1[:])

            # ---- fc2 + bias
            f2 = f2_ps.tile([10, BT], f32, name="f2", tag="f2")
            nc.tensor.matmul(f2[:], w2[:], h1[:], start=True, stop=True)
            osb = o_p.tile([10, BT], f32, name="osb", tag="osb")
            nc.scalar.activation(osb[:], f2[:], AF.Identity, bias=b2[:])

            # ---- store (feature-major; host transposes)
            nc.sync.dma_start(y_d[:, it * BT:(it + 1) * BT], osb[:])

    nc.compile()
    return nc


def _host_prep(inputs):
    x = np.ascontiguousarray(np.asarray(inputs["x"], dtype=np.float32))
    w = np.asarray(inputs["conv_w"], dtype=np.float32)
    fc1_w = np.asarray(inputs["fc1_w"], dtype=np.float32)
    fc1_b = np.asarray(inputs["fc1_b"], dtype=np.float32)
    fc2_w = np.asarray(inputs["fc2_w"], dtype=np.float32)
    fc2_b = np.asarray(inputs["fc2_b"], dtype=np.float32)

    kA = np.zeros((112, 104), np.float32)
    kB = np.zeros((56, 104), np.float32)
    kC = np.zeros((112, 52), np.float32)
    for oi in range(4):
        for oj in range(26):
            m = oi * 26 + oj
            for di in range(3):
                for dj in range(3):
                    ri, ci = oi + di, oj + dj
                    if ri < 4:
                        kA[ri * 28 + ci, m] = w[di, dj]
                    else:
                        kB[(ri - 4) * 28 + ci, m] = w[di, dj]
    for oi in range(2):
        for oj in range(26):
            m = oi * 26 + oj
            for di in range(3):
                for dj in range(3):
                    kC[(oi + di) * 28 + (oj + dj), m] = w[di, dj]

    consts = {
        "kA": kA,
        "kB": kB,
        "kC": kC,
        "w1": np.ascontiguousarray(fc1_w.T),
        "b1": np.ascontiguousarray(fc1_b.reshape(128, 1)),
        "w2": np.ascontiguousarray(fc2_w.T),
        "b2": np.ascontiguousarray(fc2_b.reshape(10, 1)),
        "iden": np.eye(128, dtype=np.float32),
    }
    in_maps = []
    for c in range(N_CORES):
        m = {"x": x[c * B_CORE:(c + 1) * B_CORE]}
        m.update(consts)
        in_maps.append(m)
    return in_maps


GBT = 2048                  # batch rows per DMA-transpose group (4 tiles)


def _build_module_v4(b_core=B_CORE, n_cores=N_CORES):
    import concourse.bass as bass
    import concourse.tile as tile
    from concourse import bacc, mybir

    f32 = mybir.dt.float32
    f32r = mybir.dt.float32r
    bf16 = mybir.dt.bfloat16
    AF = mybir.ActivationFunctionType
    nt = b_core // BT

    nc = bacc.Bacc("TRN2", target_bir_lowering=False, debug=False,
                   num_devices=n_cores)

    x_d = nc.dram_tensor("x", [b_core, 784], bf16, kind="ExternalInput").ap()
    kA_d = nc.dram_tensor("kA", [112, 104], bf16, kind="ExternalInput").ap()
    kB_d = nc.dram_tensor("kB", [56, 104], bf16, kind="ExternalInput").ap()
    kC_d = nc.dram_tensor("kC", [112, 52], bf16, kind="ExternalInput").ap()
    id_d = nc.dram_tensor("iden", [128, 128], bf16, kind="ExternalInput").ap()
    w1_d = nc.dram_tensor("w1", [676, 128], f32, kind="ExternalInput").ap()
    b1_d = nc.dram_tensor("b1", [128, 1], f32, kind="ExternalInput").ap()
    w2_d = nc.dram_tensor("w2", [128, 10], f32, kind="ExternalInput").ap()
    b2_d = nc.dram_tensor("b2", [10, 1], f32, kind="ExternalInput").ap()
    y_d = nc.dram_tensor("y", [10, b_core], f32, kind="ExternalOutput").ap()

    with tile.TileContext(nc) as tc, ExitStack() as ctx:
        const = ctx.enter_context(tc.tile_pool(name="const", bufs=1))
        xbm_p = ctx.enter_context(tc.tile_pool(name="xbm", bufs=4))
        xfm_p = ctx.enter_context(tc.tile_pool(name="xfm", bufs=21))
        h_p = ctx.enter_context(tc.tile_pool(name="h", bufs=14))
        h1_p = ctx.enter_context(tc.tile_pool(name="h1", bufs=2))
        o_p = ctx.enter_context(tc.tile_pool(name="osb", bufs=1))
        tp_ps = ctx.enter_context(tc.tile_pool(name="tp_ps", bufs=2, space="PSUM"))
        cv_ps = ctx.enter_context(tc.tile_pool(name="cv_ps", bufs=3, space="PSUM"))
        f1_ps = ctx.enter_context(tc.tile_pool(name="f1_ps", bufs=2, space="PSUM"))
        f2_ps = ctx.enter_context(tc.tile_pool(name="f2_ps", bufs=1, space="PSUM"))

        iden = const.tile([128, 128], bf16, name="iden")
        nc.sync.dma_start(iden[:], id_d)
        kA = const.tile([112, 104], bf16, name="kA")
        nc.sync.dma_start(kA[:], kA_d)
        kB = const.tile([56, 104], bf16, name="kB")
        nc.sync.dma_start(kB[:], kB_d)
        kC = const.tile([112, 52], bf16, name="kC")
        nc.sync.dma_start(kC[:], kC_d)

        def load_f32r(name, shape, src):
            stg = const.tile(shape, f32, tag=f"{name}_stg", name=f"{name}_stg")
            nc.sync.dma_start(stg[:], src)
            t = const.tile(shape, f32r, tag=name, name=name)
            nc.vector.tensor_copy(t[:], stg[:])
            return t

        w1 = []
        offs = 0
        for b in range(7):
            m = 104 if b < 6 else 52
            w1.append(load_f32r(f"w1_{b}", [m, 128], w1_d[offs:offs + m, :]))
            offs += m
        w2 = load_f32r("w2", [128, 10], w2_d)
        b1 = const.tile([128, 1], f32, name="b1")
        nc.sync.dma_start(b1[:], b1_d)
        b2 = const.tile([10, 1], f32, name="b2")
        nc.sync.dma_start(b2[:], b2_d)

        y_sb = o_p.tile([10, b_core], f32, name="y_sb")

        # Phase-separated pairs: all transposes for two batch tiles, then one
        # dense block of 42 real matmuls so the PE clock-gate releases.
        def tp_group(xbm, xfm, t, eng):
            tp = tp_ps.tile([112, BT], bf16, name="tp", tag="tp")
            for c in range(4):
                nc.tensor.transpose(tp[:, c * 128:(c + 1) * 128],
                                    xbm[:, c, 112 * t:112 * t + 112],
                                    iden[:])
            xf = xfm_p.tile([112, BT], bf16, tag="xfm", name=f"xfm{t}")
            if eng == 0:
                nc.vector.tensor_copy(xf[:], tp[:])
            else:
                nc.scalar.copy(xf[:], tp[:])
            xfm.append(xf)

        def conv_block(xfm, hs, b, eng):
            if b < 6:
                cv = cv_ps.tile([104, BT], f32, name="cv", tag="cv")
                nc.tensor.matmul(cv[:], kA[:], xfm[b][:], start=True, stop=False)
                nc.tensor.matmul(cv[:], kB[:], xfm[b + 1][0:56, :],
                                 start=False, stop=True)
                h = h_p.tile([104, BT], f32r, tag="h", name=f"h{b}")
            else:
                cv = cv_ps.tile([52, BT], f32, name="cv6", tag="cv")
                nc.tensor.matmul(cv[:], kC[:], xfm[6][:], start=True, stop=True)
                h = h_p.tile([52, BT], f32r, tag="h", name="h6")
            if eng == 0:
                nc.vector.tensor_scalar_max(h[:], cv[:], 0.0)
            else:
                nc.scalar.activation(h[:], cv[:], AF.Relu)
            hs.append(h)

        for it0 in range(0, nt, 2):
            its = [it0, it0 + 1]
            xfms = []
            hss = []
            for k, it in enumerate(its):
                xbm = xbm_p.tile([128, 4, 784], bf16, name="xbm", tag="xbm")
                src = x_d[it * BT:(it + 1) * BT, :].rearrange(
                    "(c p) d -> p c d", p=128)
                (nc.sync if it % 2 == 0 else nc.scalar).dma_start(xbm[:], src)
                xfm = []
                for t in range(7):
                    tp_group(xbm, xfm, t, (t + k) % 2)
                xfms.append(xfm)
                hss.append([])

            for k in range(2):
                for b in range(7):
                    conv_block(xfms[k], hss[k], b, (b + k) % 2)

            f1s = []
            for k, it in enumerate(its):
                f1s.append(f1_ps.tile([128, BT], f32, name=f"f1_{k}", tag="f1"))
            for b in range(7):
                for k in range(2):
                    nc.tensor.matmul(f1s[k][:], w1[b][:], hss[k][b][:],
                                     start=(b == 0), stop=(b == 6))
            for k, it in enumerate(its):
                h1 = h1_p.tile([128, BT], f32r, name=f"h1_{k}", tag="h1")
                nc.scalar.activation(h1[:], f1s[k][:], AF.Relu, bias=b1[:])
                f2 = f2_ps.tile([10, BT], f32, name=f"f2_{k}", tag="f2")
                nc.tensor.matmul(f2[:], w2[:], h1[:], start=True, stop=True)
                nc.scalar.activation(y_sb[:, it * BT:(it + 1) * BT], f2[:],
                                     AF.Identity, bias=b2[:])

        # ---- single store at the very end (feature-major; host transposes)
        nc.sync.dma_start(y_d, y_sb[:])

    nc.compile()
    return nc


def _host_prep_v4(inputs):
    import ml_dtypes
    bf = ml_dtypes.bfloat16
    x = np.asarray(inputs["x"], dtype=np.float32)
    w = np.asarray(inputs["conv_w"], dtype=np.float32)
    fc1_w = np.asarray(inputs["fc1_w"], dtype=np.float32)
    fc1_b = np.asarray(inputs["fc1_b"], dtype=np.float32)
    fc2_w = np.asarray(inputs["fc2_w"], dtype=np.float32)
    fc2_b = np.asarray(inputs["fc2_b"], dtype=np.float32)

    xp = np.ascontiguousarray(x.astype(bf))

    kA = np.zeros((112, 104), np.float32)
    kB = np.zeros((56, 104), np.float32)
    kC = np.zeros((112, 52), np.float32)
    for oi in range(4):
        for oj in range(26):
            m = oi * 26 + oj
            for di in range(3):
                for dj in range(3):
                    ri, ci = oi + di, oj + dj
                    if ri < 4:
                        kA[ri * 28 + ci, m] = w[di, dj]
                    else:
                        kB[(ri - 4) * 28 + ci, m] = w[di, dj]
    for oi in range(2):
        for oj in range(26):
            m = oi * 26 + oj
            for di in range(3):
                for dj in range(3):
                    kC[(oi + di) * 28 + (oj + dj), m] = w[di, dj]

    consts = {
        "kA": kA.astype(bf),
        "kB": kB.astype(bf),
        "kC": kC.astype(bf),
        "iden": np.eye(128, dtype=np.float32).astype(bf),
        "w1": np.ascontiguousarray(fc1_w.T),
        "b1": np.ascontiguousarray(fc1_b.reshape(128, 1)),
        "w2": np.ascontiguousarray(fc2_w.T),
        "b2": np.ascontiguousarray(fc2_b.reshape(10, 1)),
    }
    in_maps = []
    for c in range(N_CORES):
        m = {"x": xp[c * B_CORE:(c + 1) * B_CORE]}
        m.update(consts)
        in_maps.append(m)
    return in_maps



def _build_module_v6(b_core=B_CORE, n_cores=N_CORES):
    import concourse.bass as bass
    import concourse.tile as tile
    from concourse import bacc, mybir

    f32 = mybir.dt.float32
    f32r = mybir.dt.float32r
    bf16 = mybir.dt.bfloat16
    AF = mybir.ActivationFunctionType
    nt = b_core // BT

    nc = bacc.Bacc("TRN2", target_bir_lowering=False, debug=False,
                   num_devices=n_cores)

    # x arrives feature-major from the host: [7 row-groups, 112 pixels, batch]
    x_d = nc.dram_tensor("x", [7, 112, b_core], bf16, kind="ExternalInput").ap()
    kA_d = nc.dram_tensor("kA", [112, 104], bf16, kind="ExternalInput").ap()
    kB_d = nc.dram_tensor("kB", [56, 104], bf16, kind="ExternalInput").ap()
    kC_d = nc.dram_tensor("kC", [112, 52], bf16, kind="ExternalInput").ap()
    w1_d = nc.dram_tensor("w1", [676, 128], bf16, kind="ExternalInput").ap()
    b1_d = nc.dram_tensor("b1", [128, 1], f32, kind="ExternalInput").ap()
    w2_d = nc.dram_tensor("w2", [128, 10], bf16, kind="ExternalInput").ap()
    b2_d = nc.dram_tensor("b2", [10, 1], f32, kind="ExternalInput").ap()
    y_d = nc.dram_tensor("y", [10, b_core], f32, kind="ExternalOutput").ap()

    with tile.TileContext(nc) as tc, ExitStack() as ctx:
        const = ctx.enter_context(tc.tile_pool(name="const", bufs=1))
        xfm_p = ctx.enter_context(tc.tile_pool(name="xfm", bufs=3))
        h_p = ctx.enter_context(tc.tile_pool(name="h", bufs=14))
        h1_p = ctx.enter_context(tc.tile_pool(name="h1", bufs=2))
        o_p = ctx.enter_context(tc.tile_pool(name="osb", bufs=1))
        cv_ps = ctx.enter_context(tc.tile_pool(name="cv_ps", bufs=4, space="PSUM"))
        f1_ps = ctx.enter_context(tc.tile_pool(name="f1_ps", bufs=2, space="PSUM"))
        f2_ps = ctx.enter_context(tc.tile_pool(name="f2_ps", bufs=2, space="PSUM"))

        kA = const.tile([112, 104], bf16, name="kA")
        nc.sync.dma_start(kA[:], kA_d)
        kB = const.tile([56, 104], bf16, name="kB")
        nc.sync.dma_start(kB[:], kB_d)
        kC = const.tile([112, 52], bf16, name="kC")
        nc.sync.dma_start(kC[:], kC_d)

        w1 = []
        offs = 0
        for b in range(7):
            m = 104 if b < 6 else 52
            t = const.tile([m, 128], bf16, tag=f"w1_{b}", name=f"w1_{b}")
            nc.sync.dma_start(t[:], w1_d[offs:offs + m, :])
            w1.append(t)
            offs += m
        w2 = const.tile([128, 10], bf16, name="w2")
        nc.sync.dma_start(w2[:], w2_d)
        b1 = const.tile([128, 1], f32, name="b1")
        nc.sync.dma_start(b1[:], b1_d)
        b2 = const.tile([10, 1], f32, name="b2")
        nc.sync.dma_start(b2[:], b2_d)

        y_sb = o_p.tile([10, b_core], f32, name="y_sb")

        # Two batch-tiles are processed as interleaved instruction streams:
        # consecutive PE matmuls belong to independent tiles (different PSUM
        # banks, independent deps) so fill/drain phases overlap.
        def emit_pair(its):
            xfms, hss, f1s, h1s = [], [], [], []
            for k, it in enumerate(its):
                xfm = xfm_p.tile([112, 7, BT], bf16, name="xfm", tag="xfm")
                src = x_d[:, :, it * BT:(it + 1) * BT].rearrange("g p b -> p g b")
                (nc.sync if it % 2 == 0 else nc.scalar).dma_start(xfm[:], src)
                xfms.append(xfm)
                hss.append([])

            for b in range(7):
                cvs = []
                if b < 6:
                    for k in range(len(its)):
                        cv = cv_ps.tile([104, BT], f32, name="cv", tag="cv")
                        nc.tensor.matmul(cv[:], kA[:], xfms[k][:, b, :],
                                         start=True, stop=False)
                        cvs.append(cv)
                    for k in range(len(its)):
                        nc.tensor.matmul(cvs[k][:], kB[:],
                                         xfms[k][0:56, b + 1, :],
                                         start=False, stop=True)
                else:
                    for k in range(len(its)):
                        cv = cv_ps.tile([52, BT], f32, name="cv6", tag="cv")
                        nc.tensor.matmul(cv[:], kC[:], xfms[k][:, 6, :],
                                         start=True, stop=True)
                        cvs.append(cv)
                for k in range(len(its)):
                    h = h_p.tile([104 if b < 6 else 52, BT], bf16, tag="h",
                                 name=f"h{b}_{k}")
                    if (b + k) % 2 == 0:
                        nc.vector.tensor_scalar_max(h[:], cvs[k][:], 0.0)
                    else:
                        nc.scalar.activation(h[:], cvs[k][:], AF.Relu)
                    hss[k].append(h)

            for k, it in enumerate(its):
                f1s.append(f1_ps.tile([128, BT], f32, name=f"f1_{k}", tag="f1"))
            for b in range(7):
                for k in range(len(its)):
                    nc.tensor.matmul(f1s[k][:], w1[b][:], hss[k][b][:],
                                     start=(b == 0), stop=(b == 6))
            for k, it in enumerate(its):
                h1 = h1_p.tile([128, BT], bf16, name=f"h1_{k}", tag="h1")
                nc.scalar.activation(h1[:], f1s[k][:], AF.Relu, bias=b1[:])
                h1s.append(h1)
            for k, it in enumerate(its):
                f2 = f2_ps.tile([10, BT], f32, name=f"f2_{k}", tag="f2")
                nc.tensor.matmul(f2[:], w2[:], h1s[k][:], start=True, stop=True)
                nc.scalar.activation(y_sb[:, it * BT:(it + 1) * BT], f2[:],
                                     AF.Identity, bias=b2[:])

        for it2 in range(0, nt, 2):
            emit_pair([it2, it2 + 1])

        # ---- single store at the very end (feature-major; host transposes)
        nc.sync.dma_start(y_d, y_sb[:])

    nc.compile()
    return nc


def _host_prep_v6(inputs):
    import ml_dtypes
    bf = ml_dtypes.bfloat16
    x = np.asarray(inputs["x"], dtype=np.float32)
    w = np.asarray(inputs["conv_w"], dtype=np.float32)
    fc1_w = np.asarray(inputs["fc1_w"], dtype=np.float32)
    fc1_b = np.asarray(inputs["fc1_b"], dtype=np.float32)
    fc2_w = np.asarray(inputs["fc2_w"], dtype=np.float32)
    fc2_b = np.asarray(inputs["fc2_b"], dtype=np.float32)

    B = x.shape[0]
    # feature-major: [7 row-groups, 112 pixels, B]
    xT = np.ascontiguousarray(x.astype(bf).reshape(B, 7, 112).transpose(1, 2, 0))

    kA = np.zeros((112, 104), np.float32)
    kB = np.zeros((56, 104), np.float32)
    kC = np.zeros((112, 52), np.float32)
    for oi in range(4):
        for oj in range(26):
            m = oi * 26 + oj
            for di in range(3):
                for dj in range(3):
                    ri, ci = oi + di, oj + dj
                    if ri < 4:
                        kA[ri * 28 + ci, m] = w[di, dj]
                    else:
                        kB[(ri - 4) * 28 + ci, m] = w[di, dj]
    for oi in range(2):
        for oj in range(26):
            m = oi * 26 + oj
            for di in range(3):
                for dj in range(3):
                    kC[(oi + di) * 28 + (oj + dj), m] = w[di, dj]

    consts = {
        "kA": kA.astype(bf),
        "kB": kB.astype(bf),
        "kC": kC.astype(bf),
        "w1": np.ascontiguousarray(fc1_w.T.astype(bf)),
        "b1": np.ascontiguousarray(fc1_b.reshape(128, 1)),
        "w2": np.ascontiguousarray(fc2_w.T.astype(bf)),
        "b2": np.ascontiguousarray(fc2_b.reshape(10, 1)),
    }
    in_maps = []
    for c in range(N_CORES):
        m = {"x": np.ascontiguousarray(xT[:, :, c * B_CORE:(c + 1) * B_CORE])}
        m.update(consts)
        in_maps.append(m)
    return in_maps


def _build_module_v7(b_core=B_CORE, n_cores=N_CORES):
    """Feature-major end-to-end, zero PE transposes, 19 matmuls per 512-tile.

    x arrives host-transposed and host-tiled: [nt, 112, 7, BT] so each batch
    tile is one contiguous 0.8 MB DMA (112 partitions x 7 KB descriptors).
    Conv = 6 output chunks of <=116, each 2 accumulating matmuls against the
    two x row-groups its band window spans. fc1 = 6 chunks (matching the conv
    chunking), fc2 = 1. All bf16 weights/activations, fp32 PSUM.
    The PE stream is pure back-to-back real matmuls so the HAM clock gate
    warms once and stays at K=8/8 (the v4 baseline spent 63% of its span
    throttled at half clock because transposes don't count as PE activity).
    """
    import concourse.bass as bass
    import concourse.tile as tile
    from concourse import bacc, mybir

    f32 = mybir.dt.float32
    bf16 = mybir.dt.bfloat16
    AF = mybir.ActivationFunctionType
    nt = b_core // BT

    nc = bacc.Bacc("TRN2", target_bir_lowering=False, debug=False,
                   num_devices=n_cores)

    x_d = nc.dram_tensor("x", [nt, 112, 7, BT], bf16, kind="ExternalInput").ap()
    ka_d = nc.dram_tensor("ka", [112, 6, 116], bf16, kind="ExternalInput").ap()
    kb_d = nc.dram_tensor("kb", [112, 6, 116], bf16, kind="ExternalInput").ap()
    w1_d = nc.dram_tensor("w1", [116, 6, 128], bf16, kind="ExternalInput").ap()
    w2_d = nc.dram_tensor("w2", [128, 10], bf16, kind="ExternalInput").ap()
    b1_d = nc.dram_tensor("b1", [128, 1], f32, kind="ExternalInput").ap()
    y_d = nc.dram_tensor("y", [10, b_core], f32, kind="ExternalOutput").ap()

    with tile.TileContext(nc) as tc, ExitStack() as ctx:
        const = ctx.enter_context(tc.tile_pool(name="const", bufs=1))
        xfm_p = ctx.enter_context(tc.tile_pool(name="xfm", bufs=3))
        h_p = ctx.enter_context(tc.tile_pool(name="h", bufs=14))
        h1_p = ctx.enter_context(tc.tile_pool(name="h1", bufs=2))
        o_p = ctx.enter_context(tc.tile_pool(name="osb", bufs=1))
        cv_ps = ctx.enter_context(tc.tile_pool(name="cv_ps", bufs=4, space="PSUM"))
        f1_ps = ctx.enter_context(tc.tile_pool(name="f1_ps", bufs=2, space="PSUM"))
        f2_ps = ctx.enter_context(tc.tile_pool(name="f2_ps", bufs=2, space="PSUM"))

        xts = [None] * nt

        def load_x(it):
            t = xfm_p.tile([112, 7, BT], bf16, name="xfm", tag="xfm")
            (nc.sync if it % 2 == 0 else nc.gpsimd).dma_start(t[:], x_d[it])
            xts[it] = t

        load_x(0)
        ka = const.tile([112, 6, 116], bf16, name="ka")
        nc.sync.dma_start(ka[:], ka_d)
        kb = const.tile([112, 6, 116], bf16, name="kb")
        nc.sync.dma_start(kb[:], kb_d)
        w1 = const.tile([116, 6, 128], bf16, name="w1")
        nc.sync.dma_start(w1[:], w1_d)
        w2 = const.tile([128, 10], bf16, name="w2")
        nc.sync.dma_start(w2[:], w2_d)
        b1 = const.tile([128, 1], f32, name="b1")
        nc.sync.dma_start(b1[:], b1_d)
        load_x(1)

        y_sb = o_p.tile([10, b_core], f32, name="y_sb")

        def emit_conv(it):
            xfm = xts[it]
            hs = []
            for c in range(6):
                cv = cv_ps.tile([116, BT], f32, name="cv", tag="cv")
                nc.tensor.matmul(cv[:], ka[:, c, :], xfm[:, c, :],
                                 start=True, stop=False)
                nc.tensor.matmul(cv[:], kb[:, c, :], xfm[:, c + 1, :],
                                 start=False, stop=True)
                h = h_p.tile([116, BT], bf16, tag="h", name=f"h{c}")
                if c % 2 == 0:
                    nc.vector.tensor_scalar_max(h[:], cv[:], 0.0)
                else:
                    nc.scalar.activation(h[:], cv[:], AF.Relu)
                hs.append(h)
            return hs

        def emit_fc(it, hs):
            f1 = f1_ps.tile([128, BT], f32, name="f1", tag="f1")
            for c in range(6):
                nc.tensor.matmul(f1[:], w1[:, c, :], hs[c][:],
                                 start=(c == 0), stop=(c == 5))
            h1 = h1_p.tile([128, BT], bf16, name="h1", tag="h1")
            nc.scalar.activation(h1[:], f1[:], AF.Relu, bias=b1[:])
            f2 = f2_ps.tile([10, BT], f32, name="f2", tag="f2")
            nc.tensor.matmul(f2[:], w2[:], h1[:], start=True, stop=True)
            nc.vector.tensor_copy(y_sb[:, it * BT:(it + 1) * BT], f2[:])

        prev_hs = None
        for it in range(nt):
            if it + 2 < nt:
                load_x(it + 2)
            hs = emit_conv(it)
            if prev_hs is not None:
                emit_fc(it - 1, prev_hs)
            prev_hs = hs
        emit_fc(nt - 1, prev_hs)

        nc.sync.dma_start(y_d, y_sb[:])

    nc.compile()
    return nc


def _build_module_v8(b_core=B_CORE, n_cores=N_CORES):
    """v7 + pipelined prologue/epilogue and bank-conflict-free PE stream.

    - conv/fc1 weights load on the scalar HWDGE queue, concurrent with x
      tiles on the sync/gpsimd queues (v7 serialized them: first MM at 17.6us).
    - 8 dummy matmuls on a zeroed tile warm the HAM clock gate during the
      DMA prologue so real matmuls start at 2.4 GHz.
    - 2-deep pipeline: iteration i emits conv(i), fc1(i-1), fc2(i-2)
      interleaved [f2, cvA0, f1_0, cvB0, cvA1, f1_1, cvB1, ...] so no two
      adjacent PE instructions touch the same PSUM bank (consecutive
      accumulates into one bank stall the array by the drain latency).
    - y flushed to DRAM every 4 tiles instead of once at the end.
    """
    import concourse.bass as bass
    import concourse.tile as tile
    from concourse import bacc, mybir

    f32 = mybir.dt.float32
    bf16 = mybir.dt.bfloat16
    AF = mybir.ActivationFunctionType
    nt = b_core // BT

    nc = bacc.Bacc("TRN2", target_bir_lowering=False, debug=False,
                   num_devices=n_cores)

    x_d = nc.dram_tensor("x", [nt, 112, 7, BT], bf16, kind="ExternalInput").ap()
    ka_d = nc.dram_tensor("ka", [112, 6, 116], bf16, kind="ExternalInput").ap()
    kb_d = nc.dram_tensor("kb", [112, 6, 116], bf16, kind="ExternalInput").ap()
    w1_d = nc.dram_tensor("w1", [116, 6, 128], bf16, kind="ExternalInput").ap()
    w2_d = nc.dram_tensor("w2", [128, 10], bf16, kind="ExternalInput").ap()
    b1_d = nc.dram_tensor("b1", [128, 1], f32, kind="ExternalInput").ap()
    y_d = nc.dram_tensor("y", [10, b_core], f32, kind="ExternalOutput").ap()

    with tile.TileContext(nc) as tc, ExitStack() as ctx:
        const = ctx.enter_context(tc.tile_pool(name="const", bufs=1))
        xfm_p = ctx.enter_context(tc.tile_pool(name="xfm", bufs=3))
        h_p = ctx.enter_context(tc.tile_pool(name="h", bufs=14))
        h1_p = ctx.enter_context(tc.tile_pool(name="h1", bufs=3))
        o_p = ctx.enter_context(tc.tile_pool(name="osb", bufs=1))
        cv_ps = ctx.enter_context(tc.tile_pool(name="cv_ps", bufs=4, space="PSUM"))
        f1_ps = ctx.enter_context(tc.tile_pool(name="f1_ps", bufs=2, space="PSUM"))
        f2_ps = ctx.enter_context(tc.tile_pool(name="f2_ps", bufs=2, space="PSUM"))

        # PE warm-up source: a zeroed SBUF tile (values irrelevant).
        dummy = const.tile([128, BT], bf16, name="dummy")
        nc.scalar.memzero(dummy[:])

        # conv weights go FIRST on the sync HWDGE ring (16 SDMA engines);
        # the scalar-engine ring (qScalarDynamicHW) is a single-engine
        # weights queue at ~35 GB/s — fine for the fc weights needed later,
        # fatal for anything on the critical path.
        ka = const.tile([112, 6, 116], bf16, name="ka")
        nc.sync.dma_start(ka[:], ka_d)
        kb = const.tile([112, 6, 116], bf16, name="kb")
        nc.sync.dma_start(kb[:], kb_d)
        w1 = const.tile([116, 6, 128], bf16, name="w1")
        nc.scalar.dma_start(w1[:], w1_d)
        w2 = const.tile([128, 10], bf16, name="w2")
        nc.scalar.dma_start(w2[:], w2_d)
        b1 = const.tile([128, 1], f32, name="b1")
        nc.scalar.dma_start(b1[:], b1_d)

        xts = [None] * nt

        def load_x(it):
            t = xfm_p.tile([112, 7, BT], bf16, name="xfm", tag="xfm")
            (nc.sync if it % 2 == 0 else nc.gpsimd).dma_start(t[:], x_d[it])
            xts[it] = t

        # x0 split so conv chunk 0's groups land right behind ka/kb.
        x0 = xfm_p.tile([112, 7, BT], bf16, name="xfm", tag="xfm")
        nc.sync.dma_start(x0[:, 0:2, :], x_d[0, :, 0:2, :])
        nc.sync.dma_start(x0[:, 2:7, :], x_d[0, :, 2:7, :])
        xts[0] = x0
        load_x(1)

        for _ in range(10):
            wm = f2_ps.tile([10, BT], f32, name="warm", tag="f2")
            nc.tensor.matmul(wm[:], dummy[:, 0:10], dummy[:],
                             start=True, stop=True)

        y_sb = o_p.tile([10, b_core], f32, name="y_sb")

        hs_hist = [None] * nt
        h1_hist = [None] * nt

        for i in range(nt + 2):
            conv_it = i if i < nt else None
            fc1_it = i - 1 if 0 <= i - 1 < nt else None
            fc2_it = i - 2 if 0 <= i - 2 else None

            if conv_it is not None and conv_it + 2 < nt:
                load_x(conv_it + 2)

            # fc2 of tile i-2 first (its h1 has been ready for a while)
            if fc2_it is not None:
                f2 = f2_ps.tile([10, BT], f32, name="f2", tag="f2")
                nc.tensor.matmul(f2[:], w2[:], h1_hist[fc2_it][:],
                                 start=True, stop=True)
                nc.vector.tensor_copy(y_sb[:, fc2_it * BT:(fc2_it + 1) * BT],
                                      f2[:])
                # flush in 4-tile groups, 2-tile groups near the end so the
                # final DMA covers less and the tail shrinks
                flush = {3: 4, 7: 4, 11: 4, 13: 2, 15: 2}.get(fc2_it)
                if fc2_it == nt - 1 and fc2_it not in (3, 7, 11, 13, 15):
                    flush = (fc2_it % 4) + 1
                if flush:
                    lo = (fc2_it - flush + 1) * BT
                    hi = (fc2_it + 1) * BT
                    nc.sync.dma_start(y_d[:, lo:hi], y_sb[:, lo:hi])

            if fc1_it is not None:
                f1 = f1_ps.tile([128, BT], f32, name="f1", tag="f1")
                prev_hs = hs_hist[fc1_it]

            xfm = xts[conv_it] if conv_it is not None else None
            hs = []
            for c in range(6):
                cv = None
                if xfm is not None:
                    cv = cv_ps.tile([116, BT], f32, name="cv", tag="cv")
                    nc.tensor.matmul(cv[:], ka[:, c, :], xfm[:, c, :],
                                     start=True, stop=False)
                if fc1_it is not None:
                    nc.tensor.matmul(f1[:], w1[:, c, :], prev_hs[c][:],
                                     start=(c == 0), stop=(c == 5))
                if xfm is not None:
                    nc.tensor.matmul(cv[:], kb[:, c, :], xfm[:, c + 1, :],
                                     start=False, stop=True)
                    h = h_p.tile([116, BT], bf16, tag="h", name=f"h{c}")
                    if c % 2 == 0:
                        nc.vector.tensor_scalar_max(h[:], cv[:], 0.0)
                    else:
                        nc.scalar.activation(h[:], cv[:], AF.Relu)
                    hs.append(h)
            if conv_it is not None:
                hs_hist[conv_it] = hs

            if fc1_it is not None:
                h1 = h1_p.tile([128, BT], bf16, name="h1", tag="h1")
                nc.scalar.activation(h1[:], f1[:], AF.Relu, bias=b1[:])
                h1_hist[fc1_it] = h1

    nc.compile()
    return nc


_V7_BOUNDS = [0, 112, 224, 336, 448, 560, 676]


def _host_prep_v7(inputs):
    import ml_dtypes
    bf = ml_dtypes.bfloat16
    x = np.asarray(inputs["x"], dtype=np.float32)
    w = np.asarray(inputs["conv_w"], dtype=np.float32)
    fc1_w = np.asarray(inputs["fc1_w"], dtype=np.float32)
    fc1_b = np.asarray(inputs["fc1_b"], dtype=np.float32)
    fc2_w = np.asarray(inputs["fc2_w"], dtype=np.float32)

    B = x.shape[0]
    # [B, 784] -> per-tile feature-major [B/BT, 112, 7, BT]
    xr = x.astype(bf).reshape(B // BT, BT, 7, 112)
    xt = np.ascontiguousarray(xr.transpose(0, 3, 2, 1))

    ka = np.zeros((112, 6, 116), np.float32)
    kb = np.zeros((112, 6, 116), np.float32)
    for c in range(6):
        o0, o1 = _V7_BOUNDS[c], _V7_BOUNDS[c + 1]
        for m in range(o1 - o0):
            oi, oj = divmod(o0 + m, 26)
            for di in range(3):
                for dj in range(3):
                    g, p = divmod(28 * (oi + di) + (oj + dj), 112)
                    if g == c:
                        ka[p, c, m] = w[di, dj]
                    else:
                        assert g == c + 1, (c, o0 + m, g)
                        kb[p, c, m] = w[di, dj]

    w1T = fc1_w.T  # [676, 128]
    w1u = np.zeros((116, 6, 128), np.float32)
    for c in range(6):
        o0, o1 = _V7_BOUNDS[c], _V7_BOUNDS[c + 1]
        w1u[0:o1 - o0, c, :] = w1T[o0:o1, :]

    consts = {
        "ka": ka.astype(bf),
        "kb": kb.astype(bf),
        "w1": w1u.astype(bf),
        "w2": np.ascontiguousarray(fc2_w.T.astype(bf)),
        "b1": np.ascontiguousarray(fc1_b.reshape(128, 1)),
    }
    in_maps = []
    for c in range(N_CORES):
        nt = B_CORE // BT
        m = {"x": np.ascontiguousarray(xt[c * nt:(c + 1) * nt])}
        m.update(consts)
        in_maps.append(m)
    return in_maps


VERSION = 8


def run(inputs, trace=False, tmpdir=None, version=None):
    from concourse.bass_utils import run_bass_kernel_spmd

    version = VERSION if version is None else version
    key = f"nc{version}"
    builders = {8: _build_module_v8, 7: _build_module_v7, 6: _build_module_v6,
                4: _build_module_v4, 2: _build_module}
    preps = {8: _host_prep_v7, 7: _host_prep_v7, 6: _host_prep_v6,
             4: _host_prep_v4, 2: _host_prep}
    if key not in _cache:
        _cache[key] = builders[version]()
    nc = _cache[key]
    in_maps = preps[version](inputs)
    res = run_bass_kernel_spmd(nc, in_maps, list(range(N_CORES)), trace=trace,
                               tmpdir=tmpdir)
    out = np.concatenate([np.ascontiguousarray(r["y"].T) for r in res.results], axis=0)
    if version >= 7:
        # fc2 bias is folded in on the host for v7+.
        out = out + np.asarray(inputs["fc2_b"], dtype=np.float32)[None, :]
    return out, res


def kernel(**inputs) -> np.ndarray:
    out, _ = run(inputs, trace=False)
    return out



# revision 13
# speedup vs baseline: 1.0586x; 1.0345x over previous
"""Trainium2 Bass kernel for DigitConvolutionalModel.

Pipeline (per core, pure data-parallel over batch):
  x [8192, 784] --DMA--> SBUF batch-major --PE transpose--> feature-major tiles
  conv 3x3 as banded block-matmuls on PE -> relu -> fc1 (matmul) -> relu
  -> fc2 (matmul) + bias -> DMA out.

All activations live feature-major ([features, batch]) so the PE can contract
over the partition dim. The 3x3 conv is expressed as 13 small banded matmuls
per 512-batch tile using three constant band matrices built on the host from
conv_w (shift-invariant across 4-image-row blocks).
"""

import numpy as np
from contextlib import ExitStack

N_CORES = 8
B_FULL = 65536
B_CORE = B_FULL // N_CORES  # 8192
BT = 512                    # batch tile (matmul moving free dim)
NT = B_CORE // BT           # 16

_cache = {}


def _build_module(b_core=B_CORE, n_cores=N_CORES):
    import concourse.bass as bass
    import concourse.tile as tile
    from concourse import bacc, mybir

    f32 = mybir.dt.float32
    f32r = mybir.dt.float32r
    AF = mybir.ActivationFunctionType
    nt = b_core // BT

    nc = bacc.Bacc("TRN2", target_bir_lowering=False, debug=False,
                   num_devices=n_cores)

    x_d = nc.dram_tensor("x", [b_core, 784], f32, kind="ExternalInput").ap()
    kA_d = nc.dram_tensor("kA", [112, 104], f32, kind="ExternalInput").ap()
    kB_d = nc.dram_tensor("kB", [56, 104], f32, kind="ExternalInput").ap()
    kC_d = nc.dram_tensor("kC", [112, 52], f32, kind="ExternalInput").ap()
    w1_d = nc.dram_tensor("w1", [676, 128], f32, kind="ExternalInput").ap()
    b1_d = nc.dram_tensor("b1", [128, 1], f32, kind="ExternalInput").ap()
    w2_d = nc.dram_tensor("w2", [128, 10], f32, kind="ExternalInput").ap()
    b2_d = nc.dram_tensor("b2", [10, 1], f32, kind="ExternalInput").ap()
    id_d = nc.dram_tensor("iden", [128, 128], f32, kind="ExternalInput").ap()
    y_d = nc.dram_tensor("y", [10, b_core], f32, kind="ExternalOutput").ap()

    with tile.TileContext(nc) as tc, ExitStack() as ctx:
        const = ctx.enter_context(tc.tile_pool(name="const", bufs=1))
        xbm_p = ctx.enter_context(tc.tile_pool(name="xbm", bufs=4))
        xfm_p = ctx.enter_context(tc.tile_pool(name="xfm", bufs=21))
        h_p = ctx.enter_context(tc.tile_pool(name="h", bufs=14))
        h1_p = ctx.enter_context(tc.tile_pool(name="h1", bufs=2))
        o_p = ctx.enter_context(tc.tile_pool(name="osb", bufs=2))
        tp_ps = ctx.enter_context(tc.tile_pool(name="tp_ps", bufs=2, space="PSUM"))
        cv_ps = ctx.enter_context(tc.tile_pool(name="cv_ps", bufs=2, space="PSUM"))
        f1_ps = ctx.enter_context(tc.tile_pool(name="f1_ps", bufs=2, space="PSUM"))
        f2_ps = ctx.enter_context(tc.tile_pool(name="f2_ps", bufs=2, space="PSUM"))

        iden = const.tile([128, 128], f32, name="iden")
        nc.sync.dma_start(iden[:], id_d)

        def load_f32r(name, shape, src):
            stg = const.tile(shape, f32, tag=f"{name}_stg", name=f"{name}_stg")
            nc.sync.dma_start(stg[:], src)
            t = const.tile(shape, f32r, tag=name, name=name)
            nc.vector.tensor_copy(t[:], stg[:])
            return t

        kA = load_f32r("kA", [112, 104], kA_d)
        kB = load_f32r("kB", [56, 104], kB_d)
        kC = load_f32r("kC", [112, 52], kC_d)
        w1 = []
        offs = 0
        for b in range(7):
            m = 104 if b < 6 else 52
            w1.append(load_f32r(f"w1_{b}", [m, 128], w1_d[offs:offs + m, :]))
            offs += m
        w2 = load_f32r("w2", [128, 10], w2_d)
        b1 = const.tile([128, 1], f32, name="b1")
        nc.sync.dma_start(b1[:], b1_d)
        b2 = const.tile([10, 1], f32, name="b2")
        nc.sync.dma_start(b2[:], b2_d)

        for it in range(nt):
            # ---- load one batch tile, batch-major [128, 4, 784]
            xbm = xbm_p.tile([128, 4, 784], f32, name="xbm", tag="xbm")
            src = x_d[it * BT:(it + 1) * BT, :].rearrange("(c p) d -> p c d", p=128)
            (nc.sync if it % 2 == 0 else nc.scalar).dma_start(xbm[:], src)

            # ---- transpose to feature-major tiles xfm[t] = x.T rows 112t..112t+111
            xfm = []
            for t in range(7):
                tp = tp_ps.tile([112, BT], f32, name="tp", tag="tp")
                for c in range(4):
                    nc.tensor.transpose(tp[:, c * 128:(c + 1) * 128],
                                        xbm[:, c, 112 * t:112 * t + 112],
                                        iden[:])
                xf = xfm_p.tile([112, BT], f32r, tag="xfm", name=f"xfm{t}")
                if t % 2 == 0:
                    nc.vector.tensor_copy(xf[:], tp[:])
                else:
                    nc.scalar.copy(xf[:], tp[:])
                xfm.append(xf)

            # ---- conv as banded matmuls, relu into h blocks
            hs = []
            for b in range(6):
                cv = cv_ps.tile([104, BT], f32, name="cv", tag="cv")
                nc.tensor.matmul(cv[:], kA[:], xfm[b][:], start=True, stop=False)
                nc.tensor.matmul(cv[:], kB[:], xfm[b + 1][0:56, :],
                                 start=False, stop=True)
                h = h_p.tile([104, BT], f32r, tag="h", name=f"h{b}")
                if b % 2 == 0:
                    nc.vector.tensor_scalar_max(h[:], cv[:], 0.0)
                else:
                    nc.scalar.activation(h[:], cv[:], AF.Relu)
                hs.append(h)
            cv = cv_ps.tile([52, BT], f32, name="cv6", tag="cv")
            nc.tensor.matmul(cv[:], kC[:], xfm[6][:], start=True, stop=True)
            h = h_p.tile([52, BT], f32r, tag="h", name="h6")
            nc.vector.tensor_scalar_max(h[:], cv[:], 0.0)
            hs.append(h)

            # ---- fc1: accumulate 7 chunks, relu + bias
            f1 = f1_ps.tile([128, BT], f32, name="f1", tag="f1")
            for b in range(7):
                nc.tensor.matmul(f1[:], w1[b][:], hs[b][:],
                                 start=(b == 0), stop=(b == 6))
            h1 = h1_p.tile([128, BT], f32r, name="h1", tag="h1")
            nc.scalar.activation(h1[:], f1[:], AF.Relu, bias=b1[:])

            # ---- fc2 + bias
            f2 = f2_ps.tile([10, BT], f32, name="f2", tag="f2")
            nc.tensor.matmul(f2[:], w2[:], h1[:], start=True, stop=True)
            osb = o_p.tile([10, BT], f32, name="osb", tag="osb")
            nc.scalar.activation(osb[:], f2[:], AF.Identity, bias=b2[:])

            # ---- store (feature-major; host transposes)
            nc.sync.dma_start(y_d[:, it * BT:(it + 1) * BT], osb[:])

    nc.compile()
    return nc


def _host_prep(inputs):
    x = np.ascontiguousarray(np.asarray(inputs["x"], dtype=np.float32))
    w = np.asarray(inputs["conv_w"], dtype=np.float32)
    fc1_w = np.asarray(inputs["fc1_w"], dtype=np.float32)
    fc1_b = np.asarray(inputs["fc1_b"], dtype=np.float32)
    fc2_w = np.asarray(inputs["fc2_w"], dtype=np.float32)
    fc2_b = np.asarray(inputs["fc2_b"], dtype=np.float32)

    kA = np.zeros((112, 104), np.float32)
    kB = np.zeros((56, 104), np.float32)
    kC = np.zeros((112, 52), np.float32)
    for oi in range(4):
        for oj in range(26):
            m = oi * 26 + oj
            for di in range(3):
                for dj in range(3):
                    ri, ci = oi + di, oj + dj
                    if ri < 4:
                        kA[ri * 28 + ci, m] = w[di, dj]
                    else:
                        kB[(ri - 4) * 28 + ci, m] = w[di, dj]
    for oi in range(2):
        for oj in range(26):
            m = oi * 26 + oj
            for di in range(3):
                for dj in range(3):
                    kC[(oi + di) * 28 + (oj + dj), m] = w[di, dj]

    consts = {
        "kA": kA,
        "kB": kB,
        "kC": kC,
        "w1": np.ascontiguousarray(fc1_w.T),
        "b1": np.ascontiguousarray(fc1_b.reshape(128, 1)),
        "w2": np.ascontiguousarray(fc2_w.T),
        "b2": np.ascontiguousarray(fc2_b.reshape(10, 1)),
        "iden": np.eye(128, dtype=np.float32),
    }
    in_maps = []
    for c in range(N_CORES):
        m = {"x": x[c * B_CORE:(c + 1) * B_CORE]}
        m.update(consts)
        in_maps.append(m)
    return in_maps


GBT = 2048                  # batch rows per DMA-transpose group (4 tiles)


def _build_module_v4(b_core=B_CORE, n_cores=N_CORES):
    import concourse.bass as bass
    import concourse.tile as tile
    from concourse import bacc, mybir

    f32 = mybir.dt.float32
    f32r = mybir.dt.float32r
    bf16 = mybir.dt.bfloat16
    AF = mybir.ActivationFunctionType
    nt = b_core // BT

    nc = bacc.Bacc("TRN2", target_bir_lowering=False, debug=False,
                   num_devices=n_cores)

    x_d = nc.dram_tensor("x", [b_core, 784], bf16, kind="ExternalInput").ap()
    kA_d = nc.dram_tensor("kA", [112, 104], bf16, kind="ExternalInput").ap()
    kB_d = nc.dram_tensor("kB", [56, 104], bf16, kind="ExternalInput").ap()
    kC_d = nc.dram_tensor("kC", [112, 52], bf16, kind="ExternalInput").ap()
    id_d = nc.dram_tensor("iden", [128, 128], bf16, kind="ExternalInput").ap()
    w1_d = nc.dram_tensor("w1", [676, 128], f32, kind="ExternalInput").ap()
    b1_d = nc.dram_tensor("b1", [128, 1], f32, kind="ExternalInput").ap()
    w2_d = nc.dram_tensor("w2", [128, 10], f32, kind="ExternalInput").ap()
    b2_d = nc.dram_tensor("b2", [10, 1], f32, kind="ExternalInput").ap()
    y_d = nc.dram_tensor("y", [10, b_core], f32, kind="ExternalOutput").ap()

    with tile.TileContext(nc) as tc, ExitStack() as ctx:
        const = ctx.enter_context(tc.tile_pool(name="const", bufs=1))
        xbm_p = ctx.enter_context(tc.tile_pool(name="xbm", bufs=4))
        xfm_p = ctx.enter_context(tc.tile_pool(name="xfm", bufs=21))
        h_p = ctx.enter_context(tc.tile_pool(name="h", bufs=14))
        h1_p = ctx.enter_context(tc.tile_pool(name="h1", bufs=2))
        o_p = ctx.enter_context(tc.tile_pool(name="osb", bufs=1))
        tp_ps = ctx.enter_context(tc.tile_pool(name="tp_ps", bufs=2, space="PSUM"))
        cv_ps = ctx.enter_context(tc.tile_pool(name="cv_ps", bufs=3, space="PSUM"))
        f1_ps = ctx.enter_context(tc.tile_pool(name="f1_ps", bufs=2, space="PSUM"))
        f2_ps = ctx.enter_context(tc.tile_pool(name="f2_ps", bufs=1, space="PSUM"))

        iden = const.tile([128, 128], bf16, name="iden")
        nc.sync.dma_start(iden[:], id_d)
        kA = const.tile([112, 104], bf16, name="kA")
        nc.sync.dma_start(kA[:], kA_d)
        kB = const.tile([56, 104], bf16, name="kB")
        nc.sync.dma_start(kB[:], kB_d)
        kC = const.tile([112, 52], bf16, name="kC")
        nc.sync.dma_start(kC[:], kC_d)

        def load_f32r(name, shape, src):
            stg = const.tile(shape, f32, tag=f"{name}_stg", name=f"{name}_stg")
            nc.sync.dma_start(stg[:], src)
            t = const.tile(shape, f32r, tag=name, name=name)
            nc.vector.tensor_copy(t[:], stg[:])
            return t

        w1 = []
        offs = 0
        for b in range(7):
            m = 104 if b < 6 else 52
            w1.append(load_f32r(f"w1_{b}", [m, 128], w1_d[offs:offs + m, :]))
            offs += m
        w2 = load_f32r("w2", [128, 10], w2_d)
        b1 = const.tile([128, 1], f32, name="b1")
        nc.sync.dma_start(b1[:], b1_d)
        b2 = const.tile([10, 1], f32, name="b2")
        nc.sync.dma_start(b2[:], b2_d)

        y_sb = o_p.tile([10, b_core], f32, name="y_sb")

        # Phase-separated pairs: all transposes for two batch tiles, then one
        # dense block of 42 real matmuls so the PE clock-gate releases.
        def tp_group(xbm, xfm, t, eng):
            tp = tp_ps.tile([112, BT], bf16, name="tp", tag="tp")
            for c in range(4):
                nc.tensor.transpose(tp[:, c * 128:(c + 1) * 128],
                                    xbm[:, c, 112 * t:112 * t + 112],
                                    iden[:])
            xf = xfm_p.tile([112, BT], bf16, tag="xfm", name=f"xfm{t}")
            if eng == 0:
                nc.vector.tensor_copy(xf[:], tp[:])
            else:
                nc.scalar.copy(xf[:], tp[:])
            xfm.append(xf)

        def conv_block(xfm, hs, b, eng):
            if b < 6:
                cv = cv_ps.tile([104, BT], f32, name="cv", tag="cv")
                nc.tensor.matmul(cv[:], kA[:], xfm[b][:], start=True, stop=False)
                nc.tensor.matmul(cv[:], kB[:], xfm[b + 1][0:56, :],
                                 start=False, stop=True)
                h = h_p.tile([104, BT], f32r, tag="h", name=f"h{b}")
            else:
                cv = cv_ps.tile([52, BT], f32, name="cv6", tag="cv")
                nc.tensor.matmul(cv[:], kC[:], xfm[6][:], start=True, stop=True)
                h = h_p.tile([52, BT], f32r, tag="h", name="h6")
            if eng == 0:
                nc.vector.tensor_scalar_max(h[:], cv[:], 0.0)
            else:
                nc.scalar.activation(h[:], cv[:], AF.Relu)
            hs.append(h)

        for it0 in range(0, nt, 2):
            its = [it0, it0 + 1]
            xfms = []
            hss = []
            for k, it in enumerate(its):
                xbm = xbm_p.tile([128, 4, 784], bf16, name="xbm", tag="xbm")
                src = x_d[it * BT:(it + 1) * BT, :].rearrange(
                    "(c p) d -> p c d", p=128)
                (nc.sync if it % 2 == 0 else nc.scalar).dma_start(xbm[:], src)
                xfm = []
                for t in range(7):
                    tp_group(xbm, xfm, t, (t + k) % 2)
                xfms.append(xfm)
                hss.append([])

            for k in range(2):
                for b in range(7):
                    conv_block(xfms[k], hss[k], b, (b + k) % 2)

            f1s = []
            for k, it in enumerate(its):
                f1s.append(f1_ps.tile([128, BT], f32, name=f"f1_{k}", tag="f1"))
            for b in range(7):
                for k in range(2):
                    nc.tensor.matmul(f1s[k][:], w1[b][:], hss[k][b][:],
                                     start=(b == 0), stop=(b == 6))
            for k, it in enumerate(its):
                h1 = h1_p.tile([128, BT], f32r, name=f"h1_{k}", tag="h1")
                nc.scalar.activation(h1[:], f1s[k][:], AF.Relu, bias=b1[:])
                f2 = f2_ps.tile([10, BT], f32, name=f"f2_{k}", tag="f2")
                nc.tensor.matmul(f2[:], w2[:], h1[:], start=True, stop=True)
                nc.scalar.activation(y_sb[:, it * BT:(it + 1) * BT], f2[:],
                                     AF.Identity, bias=b2[:])

        # ---- single store at the very end (feature-major; host transposes)
        nc.sync.dma_start(y_d, y_sb[:])

    nc.compile()
    return nc


def _host_prep_v4(inputs):
    import ml_dtypes
    bf = ml_dtypes.bfloat16
    x = np.asarray(inputs["x"], dtype=np.float32)
    w = np.asarray(inputs["conv_w"], dtype=np.float32)
    fc1_w = np.asarray(inputs["fc1_w"], dtype=np.float32)
    fc1_b = np.asarray(inputs["fc1_b"], dtype=np.float32)
    fc2_w = np.asarray(inputs["fc2_w"], dtype=np.float32)
    fc2_b = np.asarray(inputs["fc2_b"], dtype=np.float32)

    xp = np.ascontiguousarray(x.astype(bf))

    kA = np.zeros((112, 104), np.float32)
    kB = np.zeros((56, 104), np.float32)
    kC = np.zeros((112, 52), np.float32)
    for oi in range(4):
        for oj in range(26):
            m = oi * 26 + oj
            for di in range(3):
                for dj in range(3):
                    ri, ci = oi + di, oj + dj
                    if ri < 4:
                        kA[ri * 28 + ci, m] = w[di, dj]
                    else:
                        kB[(ri - 4) * 28 + ci, m] = w[di, dj]
    for oi in range(2):
        for oj in range(26):
            m = oi * 26 + oj
            for di in range(3):
                for dj in range(3):
                    kC[(oi + di) * 28 + (oj + dj), m] = w[di, dj]

    consts = {
        "kA": kA.astype(bf),
        "kB": kB.astype(bf),
        "kC": kC.astype(bf),
        "iden": np.eye(128, dtype=np.float32).astype(bf),
        "w1": np.ascontiguousarray(fc1_w.T),
        "b1": np.ascontiguousarray(fc1_b.reshape(128, 1)),
        "w2": np.ascontiguousarray(fc2_w.T),
        "b2": np.ascontiguousarray(fc2_b.reshape(10, 1)),
    }
    in_maps = []
    for c in range(N_CORES):
        m = {"x": xp[c * B_CORE:(c + 1) * B_CORE]}
        m.update(consts)
        in_maps.append(m)
    return in_maps



def _build_module_v6(b_core=B_CORE, n_cores=N_CORES):
    import concourse.bass as bass
    import concourse.tile as tile
    from concourse import bacc, mybir

    f32 = mybir.dt.float32
    f32r = mybir.dt.float32r
    bf16 = mybir.dt.bfloat16
    AF = mybir.ActivationFunctionType
    nt = b_core // BT

    nc = bacc.Bacc("TRN2", target_bir_lowering=False, debug=False,
                   num_devices=n_cores)

    # x arrives feature-major from the host: [7 row-groups, 112 pixels, batch]
    x_d = nc.dram_tensor("x", [7, 112, b_core], bf16, kind="ExternalInput").ap()
    kA_d = nc.dram_tensor("kA", [112, 104], bf16, kind="ExternalInput").ap()
    kB_d = nc.dram_tensor("kB", [56, 104], bf16, kind="ExternalInput").ap()
    kC_d = nc.dram_tensor("kC", [112, 52], bf16, kind="ExternalInput").ap()
    w1_d = nc.dram_tensor("w1", [676, 128], bf16, kind="ExternalInput").ap()
    b1_d = nc.dram_tensor("b1", [128, 1], f32, kind="ExternalInput").ap()
    w2_d = nc.dram_tensor("w2", [128, 10], bf16, kind="ExternalInput").ap()
    b2_d = nc.dram_tensor("b2", [10, 1], f32, kind="ExternalInput").ap()
    y_d = nc.dram_tensor("y", [10, b_core], f32, kind="ExternalOutput").ap()

    with tile.TileContext(nc) as tc, ExitStack() as ctx:
        const = ctx.enter_context(tc.tile_pool(name="const", bufs=1))
        xfm_p = ctx.enter_context(tc.tile_pool(name="xfm", bufs=3))
        h_p = ctx.enter_context(tc.tile_pool(name="h", bufs=14))
        h1_p = ctx.enter_context(tc.tile_pool(name="h1", bufs=2))
        o_p = ctx.enter_context(tc.tile_pool(name="osb", bufs=1))
        cv_ps = ctx.enter_context(tc.tile_pool(name="cv_ps", bufs=4, space="PSUM"))
        f1_ps = ctx.enter_context(tc.tile_pool(name="f1_ps", bufs=2, space="PSUM"))
        f2_ps = ctx.enter_context(tc.tile_pool(name="f2_ps", bufs=2, space="PSUM"))

        kA = const.tile([112, 104], bf16, name="kA")
        nc.sync.dma_start(kA[:], kA_d)
        kB = const.tile([56, 104], bf16, name="kB")
        nc.sync.dma_start(kB[:], kB_d)
        kC = const.tile([112, 52], bf16, name="kC")
        nc.sync.dma_start(kC[:], kC_d)

        w1 = []
        offs = 0
        for b in range(7):
            m = 104 if b < 6 else 52
            t = const.tile([m, 128], bf16, tag=f"w1_{b}", name=f"w1_{b}")
            nc.sync.dma_start(t[:], w1_d[offs:offs + m, :])
            w1.append(t)
            offs += m
        w2 = const.tile([128, 10], bf16, name="w2")
        nc.sync.dma_start(w2[:], w2_d)
        b1 = const.tile([128, 1], f32, name="b1")
        nc.sync.dma_start(b1[:], b1_d)
        b2 = const.tile([10, 1], f32, name="b2")
        nc.sync.dma_start(b2[:], b2_d)

        y_sb = o_p.tile([10, b_core], f32, name="y_sb")

        # Two batch-tiles are processed as interleaved instruction streams:
        # consecutive PE matmuls belong to independent tiles (different PSUM
        # banks, independent deps) so fill/drain phases overlap.
        def emit_pair(its):
            xfms, hss, f1s, h1s = [], [], [], []
            for k, it in enumerate(its):
                xfm = xfm_p.tile([112, 7, BT], bf16, name="xfm", tag="xfm")
                src = x_d[:, :, it * BT:(it + 1) * BT].rearrange("g p b -> p g b")
                (nc.sync if it % 2 == 0 else nc.scalar).dma_start(xfm[:], src)
                xfms.append(xfm)
                hss.append([])

            for b in range(7):
                cvs = []
                if b < 6:
                    for k in range(len(its)):
                        cv = cv_ps.tile([104, BT], f32, name="cv", tag="cv")
                        nc.tensor.matmul(cv[:], kA[:], xfms[k][:, b, :],
                                         start=True, stop=False)
                        cvs.append(cv)
                    for k in range(len(its)):
                        nc.tensor.matmul(cvs[k][:], kB[:],
                                         xfms[k][0:56, b + 1, :],
                                         start=False, stop=True)
                else:
                    for k in range(len(its)):
                        cv = cv_ps.tile([52, BT], f32, name="cv6", tag="cv")
                        nc.tensor.matmul(cv[:], kC[:], xfms[k][:, 6, :],
                                         start=True, stop=True)
                        cvs.append(cv)
                for k in range(len(its)):
                    h = h_p.tile([104 if b < 6 else 52, BT], bf16, tag="h",
                                 name=f"h{b}_{k}")
                    if (b + k) % 2 == 0:
                        nc.vector.tensor_scalar_max(h[:], cvs[k][:], 0.0)
                    else:
                        nc.scalar.activation(h[:], cvs[k][:], AF.Relu)
                    hss[k].append(h)

            for k, it in enumerate(its):
                f1s.append(f1_ps.tile([128, BT], f32, name=f"f1_{k}", tag="f1"))
            for b in range(7):
                for k in range(len(its)):
                    nc.tensor.matmul(f1s[k][:], w1[b][:], hss[k][b][:],
                                     start=(b == 0), stop=(b == 6))
            for k, it in enumerate(its):
                h1 = h1_p.tile([128, BT], bf16, name=f"h1_{k}", tag="h1")
                nc.scalar.activation(h1[:], f1s[k][:], AF.Relu, bias=b1[:])
                h1s.append(h1)
            for k, it in enumerate(its):
                f2 = f2_ps.tile([10, BT], f32, name=f"f2_{k}", tag="f2")
                nc.tensor.matmul(f2[:], w2[:], h1s[k][:], start=True, stop=True)
                nc.scalar.activation(y_sb[:, it * BT:(it + 1) * BT], f2[:],
                                     AF.Identity, bias=b2[:])

        for it2 in range(0, nt, 2):
            emit_pair([it2, it2 + 1])

        # ---- single store at the very end (feature-major; host transposes)
        nc.sync.dma_start(y_d, y_sb[:])

    nc.compile()
    return nc


def _host_prep_v6(inputs):
    import ml_dtypes
    bf = ml_dtypes.bfloat16
    x = np.asarray(inputs["x"], dtype=np.float32)
    w = np.asarray(inputs["conv_w"], dtype=np.float32)
    fc1_w = np.asarray(inputs["fc1_w"], dtype=np.float32)
    fc1_b = np.asarray(inputs["fc1_b"], dtype=np.float32)
    fc2_w = np.asarray(inputs["fc2_w"], dtype=np.float32)
    fc2_b = np.asarray(inputs["fc2_b"], dtype=np.float32)

    B = x.shape[0]
    # feature-major: [7 row-groups, 112 pixels, B]
    xT = np.ascontiguousarray(x.astype(bf).reshape(B, 7, 112).transpose(1, 2, 0))

    kA = np.zeros((112, 104), np.float32)
    kB = np.zeros((56, 104), np.float32)
    kC = np.zeros((112, 52), np.float32)
    for oi in range(4):
        for oj in range(26):
            m = oi * 26 + oj
            for di in range(3):
                for dj in range(3):
                    ri, ci = oi + di, oj + dj
                    if ri < 4:
                        kA[ri * 28 + ci, m] = w[di, dj]
                    else:
                        kB[(ri - 4) * 28 + ci, m] = w[di, dj]
    for oi in range(2):
        for oj in range(26):
            m = oi * 26 + oj
            for di in range(3):
                for dj in range(3):
                    kC[(oi + di) * 28 + (oj + dj), m] = w[di, dj]

    consts = {
        "kA": kA.astype(bf),
        "kB": kB.astype(bf),
        "kC": kC.astype(bf),
        "w1": np.ascontiguousarray(fc1_w.T.astype(bf)),
        "b1": np.ascontiguousarray(fc1_b.reshape(128, 1)),
        "w2": np.ascontiguousarray(fc2_w.T.astype(bf)),
        "b2": np.ascontiguousarray(fc2_b.reshape(10, 1)),
    }
    in_maps = []
    for c in range(N_CORES):
        m = {"x": np.ascontiguousarray(xT[:, :, c * B_CORE:(c + 1) * B_CORE])}
        m.update(consts)
        in_maps.append(m)
    return in_maps


def _build_module_v7(b_core=B_CORE, n_cores=N_CORES):
    """Feature-major end-to-end, zero PE transposes, 19 matmuls per 512-tile.

    x arrives host-transposed and host-tiled: [nt, 112, 7, BT] so each batch
    tile is one contiguous 0.8 MB DMA (112 partitions x 7 KB descriptors).
    Conv = 6 output chunks of <=116, each 2 accumulating matmuls against the
    two x row-groups its band window spans. fc1 = 6 chunks (matching the conv
    chunking), fc2 = 1. All bf16 weights/activations, fp32 PSUM.
    The PE stream is pure back-to-back real matmuls so the HAM clock gate
    warms once and stays at K=8/8 (the v4 baseline spent 63% of its span
    throttled at half clock because transposes don't count as PE activity).
    """
    import concourse.bass as bass
    import concourse.tile as tile
    from concourse import bacc, mybir

    f32 = mybir.dt.float32
    bf16 = mybir.dt.bfloat16
    AF = mybir.ActivationFunctionType
    nt = b_core // BT

    nc = bacc.Bacc("TRN2", target_bir_lowering=False, debug=False,
                   num_devices=n_cores)

    x_d = nc.dram_tensor("x", [nt, 112, 7, BT], bf16, kind="ExternalInput").ap()
    ka_d = nc.dram_tensor("ka", [112, 6, 116], bf16, kind="ExternalInput").ap()
    kb_d = nc.dram_tensor("kb", [112, 6, 116], bf16, kind="ExternalInput").ap()
    w1_d = nc.dram_tensor("w1", [116, 6, 128], bf16, kind="ExternalInput").ap()
    w2_d = nc.dram_tensor("w2", [128, 10], bf16, kind="ExternalInput").ap()
    b1_d = nc.dram_tensor("b1", [128, 1], f32, kind="ExternalInput").ap()
    y_d = nc.dram_tensor("y", [10, b_core], f32, kind="ExternalOutput").ap()

    with tile.TileContext(nc) as tc, ExitStack() as ctx:
        const = ctx.enter_context(tc.tile_pool(name="const", bufs=1))
        xfm_p = ctx.enter_context(tc.tile_pool(name="xfm", bufs=3))
        h_p = ctx.enter_context(tc.tile_pool(name="h", bufs=14))
        h1_p = ctx.enter_context(tc.tile_pool(name="h1", bufs=2))
        o_p = ctx.enter_context(tc.tile_pool(name="osb", bufs=1))
        cv_ps = ctx.enter_context(tc.tile_pool(name="cv_ps", bufs=4, space="PSUM"))
        f1_ps = ctx.enter_context(tc.tile_pool(name="f1_ps", bufs=2, space="PSUM"))
        f2_ps = ctx.enter_context(tc.tile_pool(name="f2_ps", bufs=2, space="PSUM"))

        xts = [None] * nt

        def load_x(it):
            t = xfm_p.tile([112, 7, BT], bf16, name="xfm", tag="xfm")
            (nc.sync if it % 2 == 0 else nc.gpsimd).dma_start(t[:], x_d[it])
            xts[it] = t

        load_x(0)
        ka = const.tile([112, 6, 116], bf16, name="ka")
        nc.sync.dma_start(ka[:], ka_d)
        kb = const.tile([112, 6, 116], bf16, name="kb")
        nc.sync.dma_start(kb[:], kb_d)
        w1 = const.tile([116, 6, 128], bf16, name="w1")
        nc.sync.dma_start(w1[:], w1_d)
        w2 = const.tile([128, 10], bf16, name="w2")
        nc.sync.dma_start(w2[:], w2_d)
        b1 = const.tile([128, 1], f32, name="b1")
        nc.sync.dma_start(b1[:], b1_d)
        load_x(1)

        y_sb = o_p.tile([10, b_core], f32, name="y_sb")

        def emit_conv(it):
            xfm = xts[it]
            hs = []
            for c in range(6):
                cv = cv_ps.tile([116, BT], f32, name="cv", tag="cv")
                nc.tensor.matmul(cv[:], ka[:, c, :], xfm[:, c, :],
                                 start=True, stop=False)
                nc.tensor.matmul(cv[:], kb[:, c, :], xfm[:, c + 1, :],
                                 start=False, stop=True)
                h = h_p.tile([116, BT], bf16, tag="h", name=f"h{c}")
                if c % 2 == 0:
                    nc.vector.tensor_scalar_max(h[:], cv[:], 0.0)
                else:
                    nc.scalar.activation(h[:], cv[:], AF.Relu)
                hs.append(h)
            return hs

        def emit_fc(it, hs):
            f1 = f1_ps.tile([128, BT], f32, name="f1", tag="f1")
            for c in range(6):
                nc.tensor.matmul(f1[:], w1[:, c, :], hs[c][:],
                                 start=(c == 0), stop=(c == 5))
            h1 = h1_p.tile([128, BT], bf16, name="h1", tag="h1")
            nc.scalar.activation(h1[:], f1[:], AF.Relu, bias=b1[:])
            f2 = f2_ps.tile([10, BT], f32, name="f2", tag="f2")
            nc.tensor.matmul(f2[:], w2[:], h1[:], start=True, stop=True)
            nc.vector.tensor_copy(y_sb[:, it * BT:(it + 1) * BT], f2[:])

        prev_hs = None
        for it in range(nt):
            if it + 2 < nt:
                load_x(it + 2)
            hs = emit_conv(it)
            if prev_hs is not None:
                emit_fc(it - 1, prev_hs)
            prev_hs = hs
        emit_fc(nt - 1, prev_hs)

        nc.sync.dma_start(y_d, y_sb[:])

    nc.compile()
    return nc


def _build_module_v8(b_core=B_CORE, n_cores=N_CORES):
    """v7 + pipelined prologue/epilogue and bank-conflict-free PE stream.

    - conv/fc1 weights load on the scalar HWDGE queue, concurrent with x
      tiles on the sync/gpsimd queues (v7 serialized them: first MM at 17.6us).
    - 8 dummy matmuls on a zeroed tile warm the HAM clock gate during the
      DMA prologue so real matmuls start at 2.4 GHz.
    - 2-deep pipeline: iteration i emits conv(i), fc1(i-1), fc2(i-2)
      interleaved [f2, cvA0, f1_0, cvB0, cvA1, f1_1, cvB1, ...] so no two
      adjacent PE instructions touch the same PSUM bank (consecutive
      accumulates into one bank stall the array by the drain latency).
    - y flushed to DRAM every 4 tiles instead of once at the end.
    """
    import concourse.bass as bass
    import concourse.tile as tile
    from concourse import bacc, mybir

    f32 = mybir.dt.float32
    bf16 = mybir.dt.bfloat16
    AF = mybir.ActivationFunctionType
    nt = b_core // BT

    nc = bacc.Bacc("TRN2", target_bir_lowering=False, debug=False,
                   num_devices=n_cores)

    x_d = nc.dram_tensor("x", [nt, 112, 7, BT], bf16, kind="ExternalInput").ap()
    kw_d = nc.dram_tensor("kw", [128, 2172], bf16, kind="ExternalInput").ap()
    y_d = nc.dram_tensor("y", [10, b_core], f32, kind="ExternalOutput").ap()

    with tile.TileContext(nc) as tc, ExitStack() as ctx:
        const = ctx.enter_context(tc.tile_pool(name="const", bufs=1))
        xfm_p = ctx.enter_context(tc.tile_pool(name="xfm", bufs=3))
        h_p = ctx.enter_context(tc.tile_pool(name="h", bufs=14))
        h1_p = ctx.enter_context(tc.tile_pool(name="h1", bufs=3))
        o_p = ctx.enter_context(tc.tile_pool(name="osb", bufs=1))
        cv_ps = ctx.enter_context(tc.tile_pool(name="cv_ps", bufs=4, space="PSUM"))
        f1_ps = ctx.enter_context(tc.tile_pool(name="f1_ps", bufs=2, space="PSUM"))
        f2_ps = ctx.enter_context(tc.tile_pool(name="f2_ps", bufs=2, space="PSUM"))

        # PE warm-up source: a zeroed SBUF tile (values irrelevant).
        dummy = const.tile([128, BT], bf16, name="dummy")
        nc.scalar.memzero(dummy[:])

        # All weights in ONE packed blob: per-partition descriptors are
        # 4.3 KB instead of 1.4 KB, which moves at ~290 GB/s instead of
        # ~40 GB/s (small-descriptor HBM reads are descriptor-dominated).
        # Layout (bf16 columns): ka 6x116 | kb 6x116 | w1 6x128 | w2 10
        # | b1 as 2 bf16 columns bitcast back to f32.
        kw = const.tile([128, 2172], bf16, name="kw")
        nc.sync.dma_start(kw[:], kw_d)
        ka = lambda c: kw[0:112, c * 116:(c + 1) * 116]
        kb = lambda c: kw[0:112, 696 + c * 116:696 + (c + 1) * 116]
        w1 = lambda c: kw[0:116, 1392 + c * 128:1392 + (c + 1) * 128]
        w2_ap = kw[0:128, 2160:2170]
        b1_ap = kw[:, 2170:2172].bitcast(f32)

        xts = [None] * nt

        def load_x(it):
            t = xfm_p.tile([112, 7, BT], bf16, name="xfm", tag="xfm")
            (nc.sync if it % 2 == 0 else nc.gpsimd).dma_start(t[:], x_d[it])
            xts[it] = t

        # x0 split so conv chunk 0's groups land right behind the weights.
        x0 = xfm_p.tile([112, 7, BT], bf16, name="xfm", tag="xfm")
        nc.sync.dma_start(x0[:, 0:2, :], x_d[0, :, 0:2, :])
        nc.sync.dma_start(x0[:, 2:7, :], x_d[0, :, 2:7, :])
        xts[0] = x0
        load_x(1)

        # 6 warm-up matmuls across 6 distinct PSUM banks (no WAW stalls, so
        # the stream is dense enough for the HAM busy-window to fire).
        for i in range(6):
            if i % 3 == 2:
                wm = f2_ps.tile([10, BT], f32, name="warm", tag="f2")
                nc.tensor.matmul(wm[:], dummy[:, 0:10], dummy[:],
                                 start=True, stop=True)
            else:
                wm = cv_ps.tile([116, BT], f32, name="warmc", tag="cv")
                nc.tensor.matmul(wm[:], dummy[:, 0:116], dummy[:],
                                 start=True, stop=True)

        y_sb = o_p.tile([10, b_core], f32, name="y_sb")

        hs_hist = [None] * nt
        h1_hist = [None] * nt

        for i in range(nt + 2):
            conv_it = i if i < nt else None
            fc1_it = i - 1 if 0 <= i - 1 < nt else None
            fc2_it = i - 2 if 0 <= i - 2 else None

            if conv_it is not None and conv_it + 2 < nt:
                load_x(conv_it + 2)

            # fc2 of tile i-2 first (its h1 has been ready for a while)
            if fc2_it is not None:
                f2 = f2_ps.tile([10, BT], f32, name="f2", tag="f2")
                nc.tensor.matmul(f2[:], w2_ap, h1_hist[fc2_it][:],
                                 start=True, stop=True)
                nc.vector.tensor_copy(y_sb[:, fc2_it * BT:(fc2_it + 1) * BT],
                                      f2[:])
                # flush in 4-tile groups, single tiles at the end so the
                # final DMA covers less and the tail shrinks
                flush = {3: 4, 7: 4, 11: 4, 13: 2, 14: 1, 15: 1}.get(fc2_it)
                if fc2_it == nt - 1 and flush is None:
                    flush = (fc2_it % 4) + 1
                if flush:
                    lo = (fc2_it - flush + 1) * BT
                    hi = (fc2_it + 1) * BT
                    nc.sync.dma_start(y_d[:, lo:hi], y_sb[:, lo:hi])

            if fc1_it is not None:
                f1 = f1_ps.tile([128, BT], f32, name="f1", tag="f1")
                prev_hs = hs_hist[fc1_it]

            xfm = xts[conv_it] if conv_it is not None else None
            hs = []
            for c in range(6):
                cv = None
                if xfm is not None:
                    cv = cv_ps.tile([116, BT], f32, name="cv", tag="cv")
                    nc.tensor.matmul(cv[:], ka(c), xfm[:, c, :],
                                     start=True, stop=False)
                if fc1_it is not None:
                    nc.tensor.matmul(f1[:], w1(c), prev_hs[c][:],
                                     start=(c == 0), stop=(c == 5))
                if xfm is not None:
                    nc.tensor.matmul(cv[:], kb(c), xfm[:, c + 1, :],
                                     start=False, stop=True)
                    h = h_p.tile([116, BT], bf16, tag="h", name=f"h{c}")
                    if c % 2 == 0:
                        nc.vector.tensor_scalar_max(h[:], cv[:], 0.0)
                    else:
                        nc.scalar.activation(h[:], cv[:], AF.Relu)
                    hs.append(h)
            if conv_it is not None:
                hs_hist[conv_it] = hs

            if fc1_it is not None:
                h1 = h1_p.tile([128, BT], bf16, name="h1", tag="h1")
                nc.scalar.activation(h1[:], f1[:], AF.Relu, bias=b1_ap)
                h1_hist[fc1_it] = h1

    nc.compile()
    return nc


_V7_BOUNDS = [0, 112, 224, 336, 448, 560, 676]


def _host_prep_v7(inputs):
    import ml_dtypes
    bf = ml_dtypes.bfloat16
    x = np.asarray(inputs["x"], dtype=np.float32)
    w = np.asarray(inputs["conv_w"], dtype=np.float32)
    fc1_w = np.asarray(inputs["fc1_w"], dtype=np.float32)
    fc1_b = np.asarray(inputs["fc1_b"], dtype=np.float32)
    fc2_w = np.asarray(inputs["fc2_w"], dtype=np.float32)

    B = x.shape[0]
    # [B, 784] -> per-tile feature-major [B/BT, 112, 7, BT]
    xr = x.astype(bf).reshape(B // BT, BT, 7, 112)
    xt = np.ascontiguousarray(xr.transpose(0, 3, 2, 1))

    ka = np.zeros((112, 6, 116), np.float32)
    kb = np.zeros((112, 6, 116), np.float32)
    for c in range(6):
        o0, o1 = _V7_BOUNDS[c], _V7_BOUNDS[c + 1]
        for m in range(o1 - o0):
            oi, oj = divmod(o0 + m, 26)
            for di in range(3):
                for dj in range(3):
                    g, p = divmod(28 * (oi + di) + (oj + dj), 112)
                    if g == c:
                        ka[p, c, m] = w[di, dj]
                    else:
                        assert g == c + 1, (c, o0 + m, g)
                        kb[p, c, m] = w[di, dj]

    w1T = fc1_w.T  # [676, 128]
    w1u = np.zeros((116, 6, 128), np.float32)
    for c in range(6):
        o0, o1 = _V7_BOUNDS[c], _V7_BOUNDS[c + 1]
        w1u[0:o1 - o0, c, :] = w1T[o0:o1, :]

    consts = {
        "ka": ka.astype(bf),
        "kb": kb.astype(bf),
        "w1": w1u.astype(bf),
        "w2": np.ascontiguousarray(fc2_w.T.astype(bf)),
        "b1": np.ascontiguousarray(fc1_b.reshape(128, 1)),
    }
    in_maps = []
    for c in range(N_CORES):
        nt = B_CORE // BT
        m = {"x": np.ascontiguousarray(xt[c * nt:(c + 1) * nt])}
        m.update(consts)
        in_maps.append(m)
    return in_maps


def _host_prep_v8(inputs):
    import ml_dtypes
    bf = ml_dtypes.bfloat16
    x = np.asarray(inputs["x"], dtype=np.float32)
    w = np.asarray(inputs["conv_w"], dtype=np.float32)
    fc1_w = np.asarray(inputs["fc1_w"], dtype=np.float32)
    fc1_b = np.asarray(inputs["fc1_b"], dtype=np.float32)
    fc2_w = np.asarray(inputs["fc2_w"], dtype=np.float32)

    B = x.shape[0]
    xr = x.astype(bf).reshape(B // BT, BT, 7, 112)
    xt = np.ascontiguousarray(xr.transpose(0, 3, 2, 1))

    ka = np.zeros((112, 6, 116), np.float32)
    kb = np.zeros((112, 6, 116), np.float32)
    for c in range(6):
        o0, o1 = _V7_BOUNDS[c], _V7_BOUNDS[c + 1]
        for m in range(o1 - o0):
            oi, oj = divmod(o0 + m, 26)
            for di in range(3):
                for dj in range(3):
                    g, p = divmod(28 * (oi + di) + (oj + dj), 112)
                    if g == c:
                        ka[p, c, m] = w[di, dj]
                    else:
                        assert g == c + 1, (c, o0 + m, g)
                        kb[p, c, m] = w[di, dj]

    w1T = fc1_w.T
    w1u = np.zeros((116, 6, 128), np.float32)
    for c in range(6):
        o0, o1 = _V7_BOUNDS[c], _V7_BOUNDS[c + 1]
        w1u[0:o1 - o0, c, :] = w1T[o0:o1, :]

    kw = np.zeros((128, 2172), bf)
    kw[0:112, 0:696] = ka.reshape(112, 696).astype(bf)
    kw[0:112, 696:1392] = kb.reshape(112, 696).astype(bf)
    kw[0:116, 1392:2160] = w1u.reshape(116, 768).astype(bf)
    kw[0:128, 2160:2170] = fc2_w.T.astype(bf)
    kw[:, 2170:2172] = np.ascontiguousarray(
        fc1_b.reshape(128, 1)).view(np.uint16).view(bf)

    consts = {"kw": kw}
    in_maps = []
    for c in range(N_CORES):
        nt = B_CORE // BT
        m = {"x": np.ascontiguousarray(xt[c * nt:(c + 1) * nt])}
        m.update(consts)
        in_maps.append(m)
    return in_maps


VERSION = 8


def run(inputs, trace=False, tmpdir=None, version=None):
    from concourse.bass_utils import run_bass_kernel_spmd

    version = VERSION if version is None else version
    key = f"nc{version}"
    builders = {8: _build_module_v8, 7: _build_module_v7, 6: _build_module_v6,
                4: _build_module_v4, 2: _build_module}
    preps = {8: _host_prep_v8, 7: _host_prep_v7, 6: _host_prep_v6,
             4: _host_prep_v4, 2: _host_prep}
    if key not in _cache:
        _cache[key] = builders[version]()
    nc = _cache[key]
    in_maps = preps[version](inputs)
    res = run_bass_kernel_spmd(nc, in_maps, list(range(N_CORES)), trace=trace,
                               tmpdir=tmpdir)
    out = np.concatenate([np.ascontiguousarray(r["y"].T) for r in res.results], axis=0)
    if version >= 7:
        # fc2 bias is folded in on the host for v7+.
        out = out + np.asarray(inputs["fc2_b"], dtype=np.float32)[None, :]
    return out, res


def kernel(**inputs) -> np.ndarray:
    out, _ = run(inputs, trace=False)
    return out

